# revision 21
# baseline (speedup 1.0000x reference)
"""BERT-base encoder layer on 8 Trainium2 NeuronCores (Bass/Tile).

Sharding: data-parallel over batch. Full inputs [32, 512, 768] split into 8
shards of 4 batches (2048 tokens); every core runs the same NEFF on its shard
(SPMD, no collectives); host concatenates the outputs.

All GEMMs run on the PE in bf16 with fp32 PSUM accumulation; softmax and
layernorm statistics run in fp32. 1/sqrt(dk) is folded into Wq on the host.
The additive attention mask is applied by a K=1 rank-1 matmul accumulated
into the score PSUM (scores are bounded, so softmax needs no max-subtract).
The softmax normalization (1/rowsum) is folded into the PE transpose of the
probabilities by using diag(1/s) instead of the identity.
"""

import os
import numpy as np
import ml_dtypes

B, S, E, H, DK, FF = 32, 512, 768, 12, 64, 3072
NCORES = 8
BL = B // NCORES          # batches per core = 4
T = BL * S                # tokens per core = 2048
EPS = 1e-12
MASK_NEG = -87.0          # stays inside exp-table range; exp() == 0 in fp32

_CACHE = {}


def _bf(a):
    return np.ascontiguousarray(np.asarray(a, np.float32).astype(ml_dtypes.bfloat16))


def _build(flags):
    import concourse.bass as bass
    import concourse.bacc as bacc
    import concourse.mybir as mybir
    import concourse.tile as tile
    from contextlib import ExitStack

    (use_bq, use_bk, use_bv, use_bo, use_bso, use_bi, use_bout,
     use_g1, use_b1, use_g2, use_b2) = flags

    AF = mybir.ActivationFunctionType
    OP = mybir.AluOpType
    AX = mybir.AxisListType
    BF16 = mybir.dt.bfloat16
    F32 = mybir.dt.float32

    nc = bacc.Bacc("TRN2", target_bir_lowering=False)

    d_x = nc.dram_tensor("x", (T, E), BF16, kind="ExternalInput")
    d_wq = nc.dram_tensor("wq", (E, E), BF16, kind="ExternalInput")
    d_wk = nc.dram_tensor("wk", (E, E), BF16, kind="ExternalInput")
    d_wv = nc.dram_tensor("wv", (E, E), BF16, kind="ExternalInput")
    d_wo = nc.dram_tensor("wo", (E, E), BF16, kind="ExternalInput")
    d_wso = nc.dram_tensor("wso", (E, E), BF16, kind="ExternalInput")
    d_wi = nc.dram_tensor("wi", (E, FF), BF16, kind="ExternalInput")
    d_wout = nc.dram_tensor("wout", (FF, E), BF16, kind="ExternalInput")
    d_mb = nc.dram_tensor("mbias", (1, T), BF16, kind="ExternalInput")
    d_id = nc.dram_tensor("ident", (128, 128), BF16, kind="ExternalInput")
    d_ones = nc.dram_tensor("onesrow", (1, 512), BF16, kind="ExternalInput")
    # bias rows: 0=bq/8, 1=bk, 2=bv, 3=bo, 4=bso, 5=bout, 6=bi (full FF width)
    d_brow = nc.dram_tensor("brow", (7, FF), BF16, kind="ExternalInput")
    d_bic = nc.dram_tensor("bicol", (128, FF // 128), F32, kind="ExternalInput")
    # gamma1 | beta1 | gamma2 | beta2, each [128, 768] partition-broadcast
    d_gb = nc.dram_tensor("gb", (128, 4 * E), F32, kind="ExternalInput")
    d_out = nc.dram_tensor("out", (T, E), F32, kind="ExternalOutput")

    KT_E = E // 128    # 6
    NT_B = S // 128    # 4
    FT = FF // 128     # 24
    HP = H // 2        # 6

    need_gb = use_g1 or use_b1 or use_g2 or use_b2
    need_brow = use_bq or use_bk or use_bv or use_bo or use_bso or use_bout

    with ExitStack() as ctx:
        tc = ctx.enter_context(tile.TileContext(nc))

        p_mm = ctx.enter_context(tc.tile_pool(name="p_mm", bufs=6, space="PSUM"))
        p_sm = ctx.enter_context(tc.tile_pool(name="p_sm", bufs=2, space="PSUM"))

        c_pool = ctx.enter_context(tc.tile_pool(name="consts", bufs=1))
        xt_pool = ctx.enter_context(tc.tile_pool(name="xt", bufs=BL * KT_E))
        att_pool = ctx.enter_context(tc.tile_pool(name="attp", bufs=BL * KT_E + 2))

        ident = c_pool.tile_from(d_id[:, :], name="ident")
        ones = c_pool.tile_from(d_ones[:, :], name="ones")
        brow = c_pool.tile_from(d_brow[:, :], name="brow") if need_brow else None
        gb = c_pool.tile_from(d_gb[:, :], name="gb") if need_gb else None

        XT = {}    # (b, kt) -> [128, S] bf16, feature-major x
        ATT = {}   # (b, kt) -> [128, S] bf16, feature-major attention context

        # ================= superphase A: x^T, QKV, attention =================
        with ExitStack() as sa:
            a_pool = sa.enter_context(tc.tile_pool(name="a_consts", bufs=1))
            wq_pool = sa.enter_context(tc.tile_pool(name="wq", bufs=KT_E))
            wk_pool = sa.enter_context(tc.tile_pool(name="wk", bufs=KT_E))
            wv_pool = sa.enter_context(tc.tile_pool(name="wv", bufs=KT_E))
            xb_pool = sa.enter_context(tc.tile_pool(name="xb", bufs=4))
            qt_pool = sa.enter_context(tc.tile_pool(name="qt", bufs=KT_E + 6))
            kt_pool = sa.enter_context(tc.tile_pool(name="kt", bufs=KT_E + 6))
            v_pool = sa.enter_context(tc.tile_pool(name="v", bufs=NT_B + 2))
            p_pool = sa.enter_context(tc.tile_pool(name="pp", bufs=26))
            pt_pool = sa.enter_context(tc.tile_pool(name="pt", bufs=12))
            dg_pool = sa.enter_context(tc.tile_pool(name="dg", bufs=14))
            s_pool = sa.enter_context(tc.tile_pool(name="sa_s", bufs=16))

            mb = a_pool.tile_from(d_mb[:, :], name="mb")
            WQ = [wq_pool.tile_from(d_wq[k * 128:(k + 1) * 128, :], name="wqt")
                  for k in range(KT_E)]
            WK = [wk_pool.tile_from(d_wk[k * 128:(k + 1) * 128, :], name="wkt")
                  for k in range(KT_E)]
            WV = [wv_pool.tile_from(d_wv[k * 128:(k + 1) * 128, :], name="wvt")
                  for k in range(KT_E)]

            for b in range(BL):
                t0 = b * S
                # ---- x -> XT (feature-major), PE transpose ----
                for kt in range(KT_E):
                    XT[(b, kt)] = xt_pool.tile([128, S], BF16, name="xtt", tag="xt")
                for tt in range(NT_B):
                    xbt = xb_pool.tile([128, E], BF16, name="xbt", tag="xb")
                    nc.gpsimd.dma_start(
                        xbt[:, :], d_x[t0 + tt * 128:t0 + (tt + 1) * 128, :])
                    tps = [p_mm.tile([128, 512], BF16, name="xtp", tag="mm")
                           for _ in range(2)]
                    for et in range(KT_E):
                        sl = tps[et // 4][:, (et % 4) * 128:(et % 4 + 1) * 128]
                        nc.tensor.transpose(
                            sl, xbt[:, et * 128:(et + 1) * 128], ident[:, :])
                    for et in range(KT_E):
                        sl = tps[et // 4][:, (et % 4) * 128:(et % 4 + 1) * 128]
                        nc.vector.tensor_copy(
                            XT[(b, et)][:, tt * 128:(tt + 1) * 128], sl)

                # ---- Q/K projections (feature-major out) ----
                QT, KTt = [None] * KT_E, [None] * KT_E
                V = [None] * NT_B
                for Wt, dstl, pool, ub, brx, tg in (
                        (WQ, QT, qt_pool, use_bq, 0, "qt"),
                        (WK, KTt, kt_pool, use_bk, 1, "kt")):
                    for et in range(KT_E):
                        ps = p_mm.tile([128, S], F32, name="qkps", tag="mm")
                        for k in range(KT_E):
                            nc.tensor.matmul(
                                ps[:, :], Wt[k][:, et * 128:(et + 1) * 128],
                                XT[(b, k)][:, :],
                                start=(k == 0), stop=(k == KT_E - 1 and not ub))
                        if ub:
                            nc.tensor.matmul(
                                ps[:, :],
                                brow[brx:brx + 1, et * 128:(et + 1) * 128],
                                ones[0:1, 0:S], start=False, stop=True)
                        dstl[et] = pool.tile([128, S], BF16, name="qkt", tag=tg)
                        nc.vector.tensor_copy(dstl[et][:, :], ps[:, :])

                # ---- V projection (token-major out) ----
                for tt in range(NT_B):
                    V[tt] = v_pool.tile([128, E], BF16, name="vt", tag="v")
                    for ec, n in ((0, 512), (512, 256)):
                        ps = (p_mm.tile([128, 512], F32, name="vps", tag="mm")
                              if n == 512 else
                              p_sm.tile([128, 256], F32, name="vps2", tag="sm"))
                        for k in range(KT_E):
                            nc.tensor.matmul(
                                ps[:, :n], XT[(b, k)][:, tt * 128:(tt + 1) * 128],
                                WV[k][:, ec:ec + n],
                                start=(k == 0), stop=(k == KT_E - 1 and not use_bv))
                        if use_bv:
                            nc.tensor.matmul(
                                ps[:, :n], ones[0:1, 0:128],
                                brow[2:3, ec:ec + n], start=False, stop=True)
                        nc.vector.tensor_copy(V[tt][:, ec:ec + n], ps[:, :n])

                # ---- attention, per head-pair ----
                for kt in range(KT_E):
                    ATT[(b, kt)] = att_pool.tile([128, S], BF16, name="attt",
                                                 tag="attT")
                # PTn[(hp, hh)][p, jt*512 + q] = P[q, jt*128+p] / s_q
                PTn = {(hp, hh): pt_pool.tile([128, NT_B * 512], BF16,
                                              name="ptn", tag="pt")
                       for hp in range(HP) for hh in range(2)}
                # software-pipelined: group g = all 12 (hp, hh) chains of
                # q-tile g. Emit scores+exp+recip+diag of group g, but the
                # PT matmuls of group g only after group g+1's front half,
                # so the PE queue never blocks on the ACT/DVE round trip.
                front = {}   # it -> list of (hp, hh, pexp, dg)
                def emit_front(it):
                    chains = []
                    for hp in range(HP):
                        for hh in range(2):
                            o = hh * 64
                            ps = p_mm.tile([128, S], F32, name="scps", tag="mm")
                            nc.tensor.matmul(
                                ps[:, :],
                                QT[hp][o:o + 64, it * 128:(it + 1) * 128],
                                KTt[hp][o:o + 64, :], start=True, stop=False)
                            nc.tensor.matmul(
                                ps[:, :], ones[0:1, 0:128], mb[:, t0:t0 + S],
                                start=False, stop=True)
                            pexp = p_pool.tile([128, S], BF16, name="pexp",
                                               tag="p")
                            ssum = s_pool.tile([128, 1], F32, name="ssum",
                                               tag="ss", bufs=64)
                            nc.scalar.activation(pexp[:, :], ps[:, :], AF.Exp,
                                                 accum_out=ssum[:, :])
                            rs = s_pool.tile([128, 1], F32, name="rsum",
                                             tag="rs")
                            nc.vector.reciprocal(rs[:, :], ssum[:, :])
                            dg = dg_pool.tile([128, 128], BF16, name="dgt",
                                              tag="dg")
                            nc.vector.tensor_scalar_mul(dg[:, :], ident[:, :],
                                                        rs[:, :])
                            chains.append((hp, hh, pexp, dg))
                    front[it] = chains

                def emit_back(it):
                    for hp, hh, pexp, dg in front.pop(it):
                        ptp = p_mm.tile([128, 512], F32, name="ptp", tag="mm")
                        for jc in range(NT_B):
                            nc.tensor.matmul(
                                ptp[:, jc * 128:(jc + 1) * 128],
                                pexp[:, jc * 128:(jc + 1) * 128], dg[:, :],
                                start=True, stop=True)
                        srcv = ptp.rearrange("p (j c) -> p j c", j=NT_B)
                        dst = PTn[(hp, hh)].rearrange("p (j c) -> p j c",
                                                      j=NT_B)
                        nc.vector.tensor_copy(
                            dst[:, :, it * 128:(it + 1) * 128], srcv)

                emit_front(0)
                for it in range(1, NT_B):
                    emit_front(it)
                    emit_back(it - 1)
                emit_back(NT_B - 1)
                for hp in range(HP):
                    ps = p_mm.tile([128, S], F32, name="avps", tag="mm")
                    for hh in range(2):
                        o = hh * 64
                        for jt in range(NT_B):
                            nc.tensor.matmul(
                                ps[o:o + 64, :],
                                V[jt][:, hp * 128 + o:hp * 128 + o + 64],
                                PTn[(hp, hh)][:, jt * 512:(jt + 1) * 512],
                                start=(jt == 0), stop=(jt == NT_B - 1),
                                tile_position=(0, o))
                    nc.vector.tensor_copy(ATT[(b, hp)][:, :], ps[:, :])

        # ============ superphase B: O-proj, SelfOutput LN, FFN, LN ===========
        with ExitStack() as sb:
            b_pool = sb.enter_context(tc.tile_pool(name="b_consts", bufs=1))
            wo_pool = sb.enter_context(tc.tile_pool(name="wo", bufs=KT_E))
            wso_pool = sb.enter_context(tc.tile_pool(name="wso", bufs=KT_E))
            wi_pool = sb.enter_context(tc.tile_pool(name="wi", bufs=KT_E))
            wout_pool = sb.enter_context(tc.tile_pool(name="wout", bufs=FT))
            h_pool = sb.enter_context(tc.tile_pool(name="h", bufs=NT_B + 1))
            ht_pool = sb.enter_context(tc.tile_pool(name="ht", bufs=2))
            fft_pool = sb.enter_context(tc.tile_pool(name="fft", bufs=FT + 2))
            sq_pool = sb.enter_context(tc.tile_pool(name="sq", bufs=2))
            rs_pool = sb.enter_context(tc.tile_pool(name="rsd", bufs=2))
            out_pool = sb.enter_context(tc.tile_pool(name="outp", bufs=2))
            t_pool = sb.enter_context(tc.tile_pool(name="sb_s", bufs=16))

            bic = b_pool.tile_from(d_bic[:, :], name="bic") if use_bi else None
            WO = [wo_pool.tile_from(d_wo[k * 128:(k + 1) * 128, :], name="wot")
                  for k in range(KT_E)]
            WSO = [wso_pool.tile_from(d_wso[k * 128:(k + 1) * 128, :], name="wsot")
                   for k in range(KT_E)]
            WI = [wi_pool.tile_from(d_wi[k * 128:(k + 1) * 128, :], name="wit")
                  for k in range(KT_E)]
            WOUT = [wout_pool.tile_from(d_wout[f * 128:(f + 1) * 128, :],
                                        name="woutt") for f in range(FT)]

            def layernorm(chunks, h_dst, gcol, use_g, use_bb, resid=None):
                """chunks: [(psum_ap, col0, n)]; h_dst: [128, E] fp32 out.
                resid: parallel list of sbuf fp32 APs added to psum first."""
                if resid is not None:
                    rtile = rs_pool.tile([128, E], F32, name="rt", tag="rsd")
                    for (ps, c0, n), rext in zip(chunks, resid):
                        nc.vector.scalar_tensor_tensor(
                            rtile[:, c0:c0 + n], ps, 1.0, rext,
                            op0=OP.mult, op1=OP.add)
                    srcs = [(rtile[:, c0:c0 + n], c0, n) for (_, c0, n) in chunks]
                else:
                    srcs = chunks
                s1 = t_pool.tile([128, 1], F32, name="s1", tag="s1")
                s1b = t_pool.tile([128, 1], F32, name="s1b", tag="s1b")
                nc.vector.reduce_sum(s1[:, :], srcs[0][0], axis=AX.X)
                nc.vector.reduce_sum(s1b[:, :], srcs[1][0], axis=AX.X)
                mu_n = t_pool.tile([128, 1], F32, name="mun", tag="mun")
                # mu_n = -(s1 + s1b)/E
                tmp = t_pool.tile([128, 1], F32, name="tmps", tag="tmps")
                nc.vector.scalar_tensor_tensor(
                    tmp[:, :], s1[:, :], 1.0, s1b[:, :], op0=OP.mult, op1=OP.add)
                nc.vector.tensor_scalar_mul(mu_n[:, :], tmp[:, :], -1.0 / E)
                ss = t_pool.tile([128, 1], F32, name="ssa", tag="ssa", bufs=34)
                ssb = t_pool.tile([128, 1], F32, name="ssb", tag="ssb", bufs=34)
                for (src, c0, n), acc in zip(srcs, (ss, ssb)):
                    sq = sq_pool.tile([128, 512], BF16, name="sqt", tag="sq")
                    nc.scalar.activation(sq[:, :n], src, AF.Square,
                                         accum_out=acc[:, :])
                # var = (ss+ssb)/E - mu^2 ; rstd = 1/sqrt(var + eps)
                musq = t_pool.tile([128, 1], F32, name="musq", tag="musq")
                nc.vector.scalar_tensor_tensor(
                    musq[:, :], mu_n[:, :], 1.0, mu_n[:, :],
                    op0=OP.mult, op1=OP.mult)
                veps = t_pool.tile([128, 1], F32, name="veps", tag="veps")
                nc.vector.scalar_tensor_tensor(
                    veps[:, :], ss[:, :], 1.0, ssb[:, :],
                    op0=OP.mult, op1=OP.add)
                veps2 = t_pool.tile([128, 1], F32, name="veps2", tag="veps2")
                nc.vector.tensor_scalar(
                    veps2[:, :], veps[:, :], 1.0 / E, EPS,
                    op0=OP.mult, op1=OP.add)
                veps3 = t_pool.tile([128, 1], F32, name="veps3", tag="veps3")
                nc.vector.scalar_tensor_tensor(
                    veps3[:, :], musq[:, :], -1.0, veps2[:, :],
                    op0=OP.mult, op1=OP.add)
                sd = t_pool.tile([128, 1], F32, name="sd", tag="sd")
                nc.scalar.sqrt(sd[:, :], veps3[:, :])
                rstd = t_pool.tile([128, 1], F32, name="rstd", tag="rstd")
                nc.vector.reciprocal(rstd[:, :], sd[:, :])
                for (src, c0, n) in srcs:
                    nc.vector.tensor_scalar(
                        h_dst[:, c0:c0 + n], src, mu_n[:, :], rstd[:, :],
                        op0=OP.add, op1=OP.mult)
                if use_g:
                    nc.vector.scalar_tensor_tensor(
                        h_dst[:, :], h_dst[:, :], 1.0,
                        gb[:, gcol * E:(gcol + 1) * E], op0=OP.mult, op1=OP.mult)
                if use_bb:
                    nc.vector.scalar_tensor_tensor(
                        h_dst[:, :], h_dst[:, :], 1.0,
                        gb[:, (gcol + 2) * E:(gcol + 3) * E],
                        op0=OP.mult, op1=OP.add)

            for b in range(BL):
                t0 = b * S
                # ---- O-projection + residual -> xa (feature-major bf16) ----
                xa = [None] * KT_E
                for et in range(KT_E):
                    ps = p_mm.tile([128, S], F32, name="ops", tag="mm")
                    for k in range(KT_E):
                        nc.tensor.matmul(
                            ps[:, :], WO[k][:, et * 128:(et + 1) * 128],
                            ATT[(b, k)][:, :],
                            start=(k == 0), stop=(k == KT_E - 1 and not use_bo))
                    if use_bo:
                        nc.tensor.matmul(
                            ps[:, :], brow[3:4, et * 128:(et + 1) * 128],
                            ones[0:1, 0:S], start=False, stop=True)
                    xa[et] = att_pool.tile([128, S], BF16, name="xat", tag="attT")
                    nc.vector.scalar_tensor_tensor(
                        xa[et][:, :], ps[:, :], 1.0, XT[(b, et)][:, :],
                        op0=OP.mult, op1=OP.add)

                # ---- SelfOutput GEMM + LN1 -> h (token-major fp32), hT ----
                hh_t = [None] * NT_B
                hT = ht_pool.tile([128, KT_E * S], BF16, name="htt", tag="ht")

                def emit_htrans(tt):
                    tps = [p_mm.tile([128, 512], BF16, name="htp", tag="mm")
                           for _ in range(2)]
                    for et in range(KT_E):
                        sl = tps[et // 4][:, (et % 4) * 128:(et % 4 + 1) * 128]
                        nc.tensor.transpose(
                            sl, hh_t[tt][:, et * 128:(et + 1) * 128],
                            ident[:, :])
                    for et in range(KT_E):
                        sl = tps[et // 4][:, (et % 4) * 128:(et % 4 + 1) * 128]
                        nc.vector.tensor_copy(
                            hT[:, et * S + tt * 128:et * S + (tt + 1) * 128], sl)

                # skewed: h-transposes of tile tt are emitted after the
                # SO GEMM of tile tt+1, so the PE never waits on LN1
                for tt in range(NT_B):
                    ch = []
                    for ec, n in ((0, 512), (512, 256)):
                        ps = (p_mm.tile([128, 512], F32, name="sops", tag="mm")
                              if n == 512 else
                              p_mm.tile([128, 256], F32, name="sops2", tag="mm"))
                        for k in range(KT_E):
                            nc.tensor.matmul(
                                ps[:, :n], xa[k][:, tt * 128:(tt + 1) * 128],
                                WSO[k][:, ec:ec + n],
                                start=(k == 0),
                                stop=(k == KT_E - 1 and not use_bso))
                        if use_bso:
                            nc.tensor.matmul(
                                ps[:, :n], ones[0:1, 0:128],
                                brow[4:5, ec:ec + n], start=False, stop=True)
                        ch.append((ps[:, :n], ec, n))
                    hh_t[tt] = h_pool.tile([128, E], BF16, name="hht", tag="h")
                    layernorm(ch, hh_t[tt], 0, use_g1, use_b1)
                    if tt > 0:
                        emit_htrans(tt - 1)
                emit_htrans(NT_B - 1)

                # ---- FFN + LN2 ----
                for tc2 in range(2):          # 256-token chunks
                    c0 = tc2 * 256
                    ffT = [None] * FT
                    for ft in range(FT):
                        ps = p_sm.tile([128, 256], F32, name="fips", tag="sm")
                        for k in range(KT_E):
                            nc.tensor.matmul(
                                ps[:, :], WI[k][:, ft * 128:(ft + 1) * 128],
                                hT[:, k * S + c0:k * S + c0 + 256],
                                start=(k == 0), stop=(k == KT_E - 1))
                        ffT[ft] = fft_pool.tile([128, 256], BF16, name="fftt",
                                                tag="fft")
                        if use_bi:
                            nc.scalar.activation(ffT[ft][:, :], ps[:, :],
                                                 AF.Gelu,
                                                 bias=bic[:, ft:ft + 1])
                        else:
                            nc.scalar.activation(ffT[ft][:, :], ps[:, :],
                                                 AF.Gelu)
                    for ti in range(2):       # 128-token tiles within chunk
                        tt = tc2 * 2 + ti
                        ch = []
                        for ec, n in ((0, 512), (512, 256)):
                            ps = (p_mm.tile([128, 512], F32, name="wops",
                                            tag="mm") if n == 512 else
                                  p_mm.tile([128, 256], F32, name="wops2",
                                            tag="mm"))
                            for f in range(FT):
                                nc.tensor.matmul(
                                    ps[:, :n],
                                    ffT[f][:, ti * 128:(ti + 1) * 128],
                                    WOUT[f][:, ec:ec + n],
                                    start=(f == 0),
                                    stop=(f == FT - 1 and not use_bout))
                            if use_bout:
                                nc.tensor.matmul(
                                    ps[:, :n], ones[0:1, 0:128],
                                    brow[5:6, ec:ec + n], start=False,
                                    stop=True)
                            ch.append((ps[:, :n], ec, n))
                        otile = out_pool.tile([128, E], F32, name="ot",
                                              tag="outp")
                        resid = [hh_t[tt][:, ec:ec + n] for (_, ec, n) in ch]
                        layernorm(ch, otile, 1, use_g2, use_b2, resid=resid)
                        nc.gpsimd.dma_start(
                            d_out[t0 + tt * 128:t0 + (tt + 1) * 128, :],
                            otile[:, :])
    nc.compile()
    return nc


def _get_program(flags):
    key = ("prog", flags)
    if key not in _CACHE:
        _CACHE[key] = _build(flags)
    return _CACHE[key]


def kernel(x, mask, Wq, bq, Wk, bk, Wv, bv, Wo, bo,
           Wso, bso, gso, beso, Wi, bi, Wout, bout, gout, beout):
    from concourse.bass_utils import run_bass_kernel_spmd

    x = np.asarray(x, np.float32)
    mask = np.asarray(mask)
    sc = 1.0 / float(np.sqrt(np.float32(DK)))

    z = lambda a: not np.any(np.asarray(a))
    one = lambda a: bool(np.all(np.asarray(a) == 1.0))
    flags = (not z(bq), not z(bk), not z(bv), not z(bo), not z(bso),
             not z(bi), not z(bout),
             not one(gso), not z(beso), not one(gout), not z(beout))
    nc = _get_program(flags)

    wq_b = _bf(np.asarray(Wq, np.float32) * sc)
    wk_b, wv_b, wo_b, wso_b = _bf(Wk), _bf(Wv), _bf(Wo), _bf(Wso)
    wi_b, wout_b = _bf(Wi), _bf(Wout)
    identb = _bf(np.eye(128))
    onesr = _bf(np.ones((1, 512)))

    brow = np.zeros((7, FF), np.float32)
    brow[0, :E] = np.asarray(bq, np.float32) * sc
    for i, v in enumerate((bk, bv, bo, bso, bout)):
        brow[i + 1, :E] = v
    brow[6, :] = bi
    brow = _bf(brow)
    bicol = np.asarray(bi, np.float32).reshape(FF // 128, 128).T.copy()
    gbt = np.zeros((128, 4 * E), np.float32)
    for i, g in enumerate((gso, gout, beso, beout)):   # gamma1|gamma2|beta1|beta2
        gbt[:, i * E:(i + 1) * E] = np.broadcast_to(
            np.asarray(g, np.float32).reshape(1, E), (128, E))

    in_maps = []
    for c in range(NCORES):
        xs = x[c * BL:(c + 1) * BL].reshape(T, E)
        ms = np.asarray(mask[c * BL:(c + 1) * BL]).reshape(BL, S)
        mbias = _bf(np.where(ms == 0, np.float32(MASK_NEG),
                             np.float32(0.0)).reshape(1, T))
        in_maps.append({
            "x": _bf(xs), "wq": wq_b, "wk": wk_b, "wv": wv_b, "wo": wo_b,
            "wso": wso_b, "wi": wi_b, "wout": wout_b, "mbias": mbias,
            "ident": identb, "onesrow": onesr,
            "brow": brow, "bicol": bicol, "gb": gbt,
        })

    trace = os.environ.get("KERNEL_TRACE", "0") == "1"
    res = run_bass_kernel_spmd(nc, in_maps, core_ids=list(range(NCORES)),
                               trace=trace)
    if trace and res.exec_time_ns is not None:
        print(f"HW exec time: {res.exec_time_ns} ns")
        if res.instructions_and_trace is not None:
            print(f"trace: {res.instructions_and_trace[1]}")
    out = np.concatenate([r["out"].reshape(BL, S, E) for r in res.results],
                         axis=0)
    return np.ascontiguousarray(out.astype(np.float32))


# revision 22
# speedup vs baseline: 1.0044x; 1.0044x over previous
"""BERT-base encoder layer on 8 Trainium2 NeuronCores (Bass/Tile).

Sharding: data-parallel over batch. Full inputs [32, 512, 768] split into 8
shards of 4 batches (2048 tokens); every core runs the same NEFF on its shard
(SPMD, no collectives); host concatenates the outputs.

All GEMMs run on the PE in bf16 with fp32 PSUM accumulation; softmax and
layernorm statistics run in fp32. 1/sqrt(dk) is folded into Wq on the host.
The additive attention mask is applied by a K=1 rank-1 matmul accumulated
into the score PSUM (scores are bounded, so softmax needs no max-subtract).
The softmax normalization (1/rowsum) is folded into the PE transpose of the
probabilities by using diag(1/s) instead of the identity.
"""

import os
import numpy as np
import ml_dtypes

B, S, E, H, DK, FF = 32, 512, 768, 12, 64, 3072
NCORES = 8
BL = B // NCORES          # batches per core = 4
T = BL * S                # tokens per core = 2048
EPS = 1e-12
MASK_NEG = -87.0          # stays inside exp-table range; exp() == 0 in fp32

_CACHE = {}


def _bf(a):
    return np.ascontiguousarray(np.asarray(a, np.float32).astype(ml_dtypes.bfloat16))


def _build(flags):
    import concourse.bass as bass
    import concourse.bacc as bacc
    import concourse.mybir as mybir
    import concourse.tile as tile
    from contextlib import ExitStack

    (use_bq, use_bk, use_bv, use_bo, use_bso, use_bi, use_bout,
     use_g1, use_b1, use_g2, use_b2) = flags

    AF = mybir.ActivationFunctionType
    OP = mybir.AluOpType
    AX = mybir.AxisListType
    BF16 = mybir.dt.bfloat16
    F32 = mybir.dt.float32

    nc = bacc.Bacc("TRN2", target_bir_lowering=False)

    d_x = nc.dram_tensor("x", (T, E), BF16, kind="ExternalInput")
    d_wq = nc.dram_tensor("wq", (E, E), BF16, kind="ExternalInput")
    d_wk = nc.dram_tensor("wk", (E, E), BF16, kind="ExternalInput")
    d_wv = nc.dram_tensor("wv", (E, E), BF16, kind="ExternalInput")
    d_wo = nc.dram_tensor("wo", (E, E), BF16, kind="ExternalInput")
    d_wso = nc.dram_tensor("wso", (E, E), BF16, kind="ExternalInput")
    d_wi = nc.dram_tensor("wi", (E, FF), BF16, kind="ExternalInput")
    d_wout = nc.dram_tensor("wout", (FF, E), BF16, kind="ExternalInput")
    d_mb = nc.dram_tensor("mbias", (1, T), BF16, kind="ExternalInput")
    d_id = nc.dram_tensor("ident", (128, 128), BF16, kind="ExternalInput")
    d_ones = nc.dram_tensor("onesrow", (1, 512), BF16, kind="ExternalInput")
    # bias rows: 0=bq/8, 1=bk, 2=bv, 3=bo, 4=bso, 5=bout, 6=bi (full FF width)
    d_brow = nc.dram_tensor("brow", (7, FF), BF16, kind="ExternalInput")
    d_bic = nc.dram_tensor("bicol", (128, FF // 128), F32, kind="ExternalInput")
    # gamma1 | beta1 | gamma2 | beta2, each [128, 768] partition-broadcast
    d_gb = nc.dram_tensor("gb", (128, 4 * E), F32, kind="ExternalInput")
    d_out = nc.dram_tensor("out", (T, E), F32, kind="ExternalOutput")

    KT_E = E // 128    # 6
    NT_B = S // 128    # 4
    FT = FF // 128     # 24
    HP = H // 2        # 6

    need_gb = use_g1 or use_b1 or use_g2 or use_b2
    need_brow = use_bq or use_bk or use_bv or use_bo or use_bso or use_bout

    with ExitStack() as ctx:
        tc = ctx.enter_context(tile.TileContext(nc))

        p_mm = ctx.enter_context(tc.tile_pool(name="p_mm", bufs=6, space="PSUM"))
        p_sm = ctx.enter_context(tc.tile_pool(name="p_sm", bufs=2, space="PSUM"))

        c_pool = ctx.enter_context(tc.tile_pool(name="consts", bufs=1))
        xt_pool = ctx.enter_context(tc.tile_pool(name="xt", bufs=BL * KT_E))
        att_pool = ctx.enter_context(tc.tile_pool(name="attp", bufs=BL * KT_E + 2))

        ident = c_pool.tile_from(d_id[:, :], name="ident")
        ones = c_pool.tile_from(d_ones[:, :], name="ones")
        brow = c_pool.tile_from(d_brow[:, :], name="brow") if need_brow else None
        gb = c_pool.tile_from(d_gb[:, :], name="gb") if need_gb else None

        XT = {}    # (b, kt) -> [128, S] bf16, feature-major x
        ATT = {}   # (b, kt) -> [128, S] bf16, feature-major attention context

        # ================= superphase A: x^T, QKV, attention =================
        with ExitStack() as sa:
            a_pool = sa.enter_context(tc.tile_pool(name="a_consts", bufs=1))
            wq_pool = sa.enter_context(tc.tile_pool(name="wq", bufs=KT_E))
            wk_pool = sa.enter_context(tc.tile_pool(name="wk", bufs=KT_E))
            wv_pool = sa.enter_context(tc.tile_pool(name="wv", bufs=KT_E))
            xb_pool = sa.enter_context(tc.tile_pool(name="xb", bufs=4))
            qt_pool = sa.enter_context(tc.tile_pool(name="qt", bufs=KT_E + 6))
            kt_pool = sa.enter_context(tc.tile_pool(name="kt", bufs=KT_E + 6))
            v_pool = sa.enter_context(tc.tile_pool(name="v", bufs=NT_B + 2))
            p_pool = sa.enter_context(tc.tile_pool(name="pp", bufs=26))
            pt_pool = sa.enter_context(tc.tile_pool(name="pt", bufs=12))
            dg_pool = sa.enter_context(tc.tile_pool(name="dg", bufs=14))
            s_pool = sa.enter_context(tc.tile_pool(name="sa_s", bufs=16))

            mb = a_pool.tile_from(d_mb[:, :], name="mb")
            WQ = [wq_pool.tile_from(d_wq[k * 128:(k + 1) * 128, :], name="wqt")
                  for k in range(KT_E)]
            WK = [wk_pool.tile_from(d_wk[k * 128:(k + 1) * 128, :], name="wkt")
                  for k in range(KT_E)]
            WV = [wv_pool.tile_from(d_wv[k * 128:(k + 1) * 128, :], name="wvt")
                  for k in range(KT_E)]

            for b in range(BL):
                t0 = b * S
                # ---- x -> XT (feature-major), PE transpose ----
                for kt in range(KT_E):
                    XT[(b, kt)] = xt_pool.tile([128, S], BF16, name="xtt", tag="xt")
                for tt in range(NT_B):
                    xbt = xb_pool.tile([128, E], BF16, name="xbt", tag="xb")
                    nc.gpsimd.dma_start(
                        xbt[:, :], d_x[t0 + tt * 128:t0 + (tt + 1) * 128, :])
                    tps = [p_mm.tile([128, 512], BF16, name="xtp", tag="mm")
                           for _ in range(2)]
                    for et in range(KT_E):
                        sl = tps[et // 4][:, (et % 4) * 128:(et % 4 + 1) * 128]
                        nc.tensor.transpose(
                            sl, xbt[:, et * 128:(et + 1) * 128], ident[:, :])
                    for et in range(KT_E):
                        sl = tps[et // 4][:, (et % 4) * 128:(et % 4 + 1) * 128]
                        nc.vector.tensor_copy(
                            XT[(b, et)][:, tt * 128:(tt + 1) * 128], sl)

                # ---- Q/K projections (feature-major out) ----
                QT, KTt = [None] * KT_E, [None] * KT_E
                V = [None] * NT_B
                for Wt, dstl, pool, ub, brx, tg in (
                        (WQ, QT, qt_pool, use_bq, 0, "qt"),
                        (WK, KTt, kt_pool, use_bk, 1, "kt")):
                    for et in range(KT_E):
                        ps = p_mm.tile([128, S], F32, name="qkps", tag="mm")
                        for k in range(KT_E):
                            nc.tensor.matmul(
                                ps[:, :], Wt[k][:, et * 128:(et + 1) * 128],
                                XT[(b, k)][:, :],
                                start=(k == 0), stop=(k == KT_E - 1 and not ub))
                        if ub:
                            nc.tensor.matmul(
                                ps[:, :],
                                brow[brx:brx + 1, et * 128:(et + 1) * 128],
                                ones[0:1, 0:S], start=False, stop=True)
                        dstl[et] = pool.tile([128, S], BF16, name="qkt", tag=tg)
                        nc.vector.tensor_copy(dstl[et][:, :], ps[:, :])

                # ---- V projection (token-major out) ----
                for tt in range(NT_B):
                    V[tt] = v_pool.tile([128, E], BF16, name="vt", tag="v")
                    for ec, n in ((0, 512), (512, 256)):
                        ps = (p_mm.tile([128, 512], F32, name="vps", tag="mm")
                              if n == 512 else
                              p_sm.tile([128, 256], F32, name="vps2", tag="sm"))
                        for k in range(KT_E):
                            nc.tensor.matmul(
                                ps[:, :n], XT[(b, k)][:, tt * 128:(tt + 1) * 128],
                                WV[k][:, ec:ec + n],
                                start=(k == 0), stop=(k == KT_E - 1 and not use_bv))
                        if use_bv:
                            nc.tensor.matmul(
                                ps[:, :n], ones[0:1, 0:128],
                                brow[2:3, ec:ec + n], start=False, stop=True)
                        nc.vector.tensor_copy(V[tt][:, ec:ec + n], ps[:, :n])

                # ---- attention, per head-pair ----
                for kt in range(KT_E):
                    ATT[(b, kt)] = att_pool.tile([128, S], BF16, name="attt",
                                                 tag="attT")
                # PTn[(hp, hh)][p, jt*512 + q] = P[q, jt*128+p] / s_q
                PTn = {(hp, hh): pt_pool.tile([128, NT_B * 512], BF16,
                                              name="ptn", tag="pt")
                       for hp in range(HP) for hh in range(2)}
                # software-pipelined: group g = all 12 (hp, hh) chains of
                # q-tile g. Emit scores+exp+recip+diag of group g, but the
                # PT matmuls of group g only after group g+1's front half,
                # so the PE queue never blocks on the ACT/DVE round trip.
                front = {}   # it -> list of (hp, hh, pexp, dg)
                def emit_front(it):
                    chains = []
                    for hp in range(HP):
                        for hh in range(2):
                            o = hh * 64
                            ps = p_mm.tile([128, S], F32, name="scps", tag="mm")
                            nc.tensor.matmul(
                                ps[:, :],
                                QT[hp][o:o + 64, it * 128:(it + 1) * 128],
                                KTt[hp][o:o + 64, :], start=True, stop=False)
                            nc.tensor.matmul(
                                ps[:, :], ones[0:1, 0:128], mb[:, t0:t0 + S],
                                start=False, stop=True)
                            pexp = p_pool.tile([128, S], BF16, name="pexp",
                                               tag="p")
                            ssum = s_pool.tile([128, 1], F32, name="ssum",
                                               tag="ss", bufs=64)
                            nc.scalar.activation(pexp[:, :], ps[:, :], AF.Exp,
                                                 accum_out=ssum[:, :])
                            rs = s_pool.tile([128, 1], F32, name="rsum",
                                             tag="rs")
                            nc.vector.reciprocal(rs[:, :], ssum[:, :])
                            dg = dg_pool.tile([128, 128], BF16, name="dgt",
                                              tag="dg")
                            nc.vector.tensor_scalar_mul(dg[:, :], ident[:, :],
                                                        rs[:, :])
                            chains.append((hp, hh, pexp, dg))
                    front[it] = chains

                def emit_back(it):
                    for hp, hh, pexp, dg in front.pop(it):
                        ptp = p_mm.tile([128, 512], F32, name="ptp", tag="mm")
                        for jc in range(NT_B):
                            nc.tensor.matmul(
                                ptp[:, jc * 128:(jc + 1) * 128],
                                pexp[:, jc * 128:(jc + 1) * 128], dg[:, :],
                                start=True, stop=True)
                        srcv = ptp.rearrange("p (j c) -> p j c", j=NT_B)
                        dst = PTn[(hp, hh)].rearrange("p (j c) -> p j c",
                                                      j=NT_B)
                        nc.vector.tensor_copy(
                            dst[:, :, it * 128:(it + 1) * 128], srcv)

                emit_front(0)
                for it in range(1, NT_B):
                    emit_front(it)
                    emit_back(it - 1)
                emit_back(NT_B - 1)
                for hp in range(HP):
                    ps = p_mm.tile([128, S], F32, name="avps", tag="mm")
                    for hh in range(2):
                        o = hh * 64
                        for jt in range(NT_B):
                            nc.tensor.matmul(
                                ps[o:o + 64, :],
                                V[jt][:, hp * 128 + o:hp * 128 + o + 64],
                                PTn[(hp, hh)][:, jt * 512:(jt + 1) * 512],
                                start=(jt == 0), stop=(jt == NT_B - 1),
                                tile_position=(0, o))
                    nc.vector.tensor_copy(ATT[(b, hp)][:, :], ps[:, :])

        # ============ superphase B: O-proj, SelfOutput LN, FFN, LN ===========
        with ExitStack() as sb:
            b_pool = sb.enter_context(tc.tile_pool(name="b_consts", bufs=1))
            wo_pool = sb.enter_context(tc.tile_pool(name="wo", bufs=KT_E))
            wso_pool = sb.enter_context(tc.tile_pool(name="wso", bufs=KT_E))
            wi_pool = sb.enter_context(tc.tile_pool(name="wi", bufs=KT_E))
            wout_pool = sb.enter_context(tc.tile_pool(name="wout", bufs=FT))
            h_pool = sb.enter_context(tc.tile_pool(name="h", bufs=NT_B + 1))
            ht_pool = sb.enter_context(tc.tile_pool(name="ht", bufs=2))
            fft_pool = sb.enter_context(tc.tile_pool(name="fft", bufs=FT + 10))
            sq_pool = sb.enter_context(tc.tile_pool(name="sq", bufs=2))
            rs_pool = sb.enter_context(tc.tile_pool(name="rsd", bufs=2))
            out_pool = sb.enter_context(tc.tile_pool(name="outp", bufs=2))
            t_pool = sb.enter_context(tc.tile_pool(name="sb_s", bufs=16))

            bic = b_pool.tile_from(d_bic[:, :], name="bic") if use_bi else None
            WO = [wo_pool.tile_from(d_wo[k * 128:(k + 1) * 128, :], name="wot")
                  for k in range(KT_E)]
            WSO = [wso_pool.tile_from(d_wso[k * 128:(k + 1) * 128, :], name="wsot")
                   for k in range(KT_E)]
            WI = [wi_pool.tile_from(d_wi[k * 128:(k + 1) * 128, :], name="wit")
                  for k in range(KT_E)]
            WOUT = [wout_pool.tile_from(d_wout[f * 128:(f + 1) * 128, :],
                                        name="woutt") for f in range(FT)]

            def layernorm(chunks, h_dst, gcol, use_g, use_bb, resid=None):
                """chunks: [(psum_ap, col0, n)]; h_dst: [128, E] fp32 out.
                resid: parallel list of sbuf fp32 APs added to psum first."""
                if resid is not None:
                    rtile = rs_pool.tile([128, E], F32, name="rt", tag="rsd")
                    for (ps, c0, n), rext in zip(chunks, resid):
                        nc.vector.scalar_tensor_tensor(
                            rtile[:, c0:c0 + n], ps, 1.0, rext,
                            op0=OP.mult, op1=OP.add)
                    srcs = [(rtile[:, c0:c0 + n], c0, n) for (_, c0, n) in chunks]
                else:
                    srcs = chunks
                s1 = t_pool.tile([128, 1], F32, name="s1", tag="s1")
                s1b = t_pool.tile([128, 1], F32, name="s1b", tag="s1b")
                nc.vector.reduce_sum(s1[:, :], srcs[0][0], axis=AX.X)
                nc.vector.reduce_sum(s1b[:, :], srcs[1][0], axis=AX.X)
                mu_n = t_pool.tile([128, 1], F32, name="mun", tag="mun")
                # mu_n = -(s1 + s1b)/E
                tmp = t_pool.tile([128, 1], F32, name="tmps", tag="tmps")
                nc.vector.scalar_tensor_tensor(
                    tmp[:, :], s1[:, :], 1.0, s1b[:, :], op0=OP.mult, op1=OP.add)
                nc.vector.tensor_scalar_mul(mu_n[:, :], tmp[:, :], -1.0 / E)
                ss = t_pool.tile([128, 1], F32, name="ssa", tag="ssa", bufs=34)
                ssb = t_pool.tile([128, 1], F32, name="ssb", tag="ssb", bufs=34)
                for (src, c0, n), acc in zip(srcs, (ss, ssb)):
                    sq = sq_pool.tile([128, 512], BF16, name="sqt", tag="sq")
                    nc.scalar.activation(sq[:, :n], src, AF.Square,
                                         accum_out=acc[:, :])
                # var = (ss+ssb)/E - mu^2 ; rstd = 1/sqrt(var + eps)
                musq = t_pool.tile([128, 1], F32, name="musq", tag="musq")
                nc.vector.scalar_tensor_tensor(
                    musq[:, :], mu_n[:, :], 1.0, mu_n[:, :],
                    op0=OP.mult, op1=OP.mult)
                veps = t_pool.tile([128, 1], F32, name="veps", tag="veps")
                nc.vector.scalar_tensor_tensor(
                    veps[:, :], ss[:, :], 1.0, ssb[:, :],
                    op0=OP.mult, op1=OP.add)
                veps2 = t_pool.tile([128, 1], F32, name="veps2", tag="veps2")
                nc.vector.tensor_scalar(
                    veps2[:, :], veps[:, :], 1.0 / E, EPS,
                    op0=OP.mult, op1=OP.add)
                veps3 = t_pool.tile([128, 1], F32, name="veps3", tag="veps3")
                nc.vector.scalar_tensor_tensor(
                    veps3[:, :], musq[:, :], -1.0, veps2[:, :],
                    op0=OP.mult, op1=OP.add)
                sd = t_pool.tile([128, 1], F32, name="sd", tag="sd")
                nc.scalar.sqrt(sd[:, :], veps3[:, :])
                rstd = t_pool.tile([128, 1], F32, name="rstd", tag="rstd")
                nc.vector.reciprocal(rstd[:, :], sd[:, :])
                for (src, c0, n) in srcs:
                    nc.vector.tensor_scalar(
                        h_dst[:, c0:c0 + n], src, mu_n[:, :], rstd[:, :],
                        op0=OP.add, op1=OP.mult)
                if use_g:
                    nc.vector.scalar_tensor_tensor(
                        h_dst[:, :], h_dst[:, :], 1.0,
                        gb[:, gcol * E:(gcol + 1) * E], op0=OP.mult, op1=OP.mult)
                if use_bb:
                    nc.vector.scalar_tensor_tensor(
                        h_dst[:, :], h_dst[:, :], 1.0,
                        gb[:, (gcol + 2) * E:(gcol + 3) * E],
                        op0=OP.mult, op1=OP.add)

            for b in range(BL):
                t0 = b * S
                # ---- O-projection + residual -> xa (feature-major bf16) ----
                xa = [None] * KT_E
                for et in range(KT_E):
                    ps = p_mm.tile([128, S], F32, name="ops", tag="mm")
                    for k in range(KT_E):
                        nc.tensor.matmul(
                            ps[:, :], WO[k][:, et * 128:(et + 1) * 128],
                            ATT[(b, k)][:, :],
                            start=(k == 0), stop=(k == KT_E - 1 and not use_bo))
                    if use_bo:
                        nc.tensor.matmul(
                            ps[:, :], brow[3:4, et * 128:(et + 1) * 128],
                            ones[0:1, 0:S], start=False, stop=True)
                    xa[et] = att_pool.tile([128, S], BF16, name="xat", tag="attT")
                    nc.vector.scalar_tensor_tensor(
                        xa[et][:, :], ps[:, :], 1.0, XT[(b, et)][:, :],
                        op0=OP.mult, op1=OP.add)

                # ---- SelfOutput GEMM + LN1 -> h (token-major fp32), hT ----
                hh_t = [None] * NT_B
                hT = ht_pool.tile([128, KT_E * S], BF16, name="htt", tag="ht")

                def emit_htrans(tt):
                    tps = [p_mm.tile([128, 512], BF16, name="htp", tag="mm")
                           for _ in range(2)]
                    for et in range(KT_E):
                        sl = tps[et // 4][:, (et % 4) * 128:(et % 4 + 1) * 128]
                        nc.tensor.transpose(
                            sl, hh_t[tt][:, et * 128:(et + 1) * 128],
                            ident[:, :])
                    for et in range(KT_E):
                        sl = tps[et // 4][:, (et % 4) * 128:(et % 4 + 1) * 128]
                        nc.vector.tensor_copy(
                            hT[:, et * S + tt * 128:et * S + (tt + 1) * 128], sl)

                # skewed: h-transposes of tile tt are emitted after the
                # SO GEMM of tile tt+1, so the PE never waits on LN1
                for tt in range(NT_B):
                    ch = []
                    for ec, n in ((0, 512), (512, 256)):
                        ps = (p_mm.tile([128, 512], F32, name="sops", tag="mm")
                              if n == 512 else
                              p_mm.tile([128, 256], F32, name="sops2", tag="mm"))
                        for k in range(KT_E):
                            nc.tensor.matmul(
                                ps[:, :n], xa[k][:, tt * 128:(tt + 1) * 128],
                                WSO[k][:, ec:ec + n],
                                start=(k == 0),
                                stop=(k == KT_E - 1 and not use_bso))
                        if use_bso:
                            nc.tensor.matmul(
                                ps[:, :n], ones[0:1, 0:128],
                                brow[4:5, ec:ec + n], start=False, stop=True)
                        ch.append((ps[:, :n], ec, n))
                    hh_t[tt] = h_pool.tile([128, E], BF16, name="hht", tag="h")
                    layernorm(ch, hh_t[tt], 0, use_g1, use_b1)
                    if tt > 0:
                        emit_htrans(tt - 1)
                emit_htrans(NT_B - 1)

                # ---- FFN + LN2 ----
                for tc2 in range(2):          # 256-token chunks
                    c0 = tc2 * 256
                    ffT = [None] * FT
                    for ft in range(FT):
                        ps = p_sm.tile([128, 256], F32, name="fips", tag="sm")
                        for k in range(KT_E):
                            nc.tensor.matmul(
                                ps[:, :], WI[k][:, ft * 128:(ft + 1) * 128],
                                hT[:, k * S + c0:k * S + c0 + 256],
                                start=(k == 0), stop=(k == KT_E - 1))
                        ffT[ft] = fft_pool.tile([128, 256], BF16, name="fftt",
                                                tag="fft")
                        if use_bi:
                            nc.scalar.activation(ffT[ft][:, :], ps[:, :],
                                                 AF.Gelu,
                                                 bias=bic[:, ft:ft + 1])
                        else:
                            nc.scalar.activation(ffT[ft][:, :], ps[:, :],
                                                 AF.Gelu)
                    for ti in range(2):       # 128-token tiles within chunk
                        tt = tc2 * 2 + ti
                        ch = []
                        for ec, n in ((0, 512), (512, 256)):
                            ps = (p_mm.tile([128, 512], F32, name="wops",
                                            tag="mm") if n == 512 else
                                  p_mm.tile([128, 256], F32, name="wops2",
                                            tag="mm"))
                            for f in range(FT):
                                nc.tensor.matmul(
                                    ps[:, :n],
                                    ffT[f][:, ti * 128:(ti + 1) * 128],
                                    WOUT[f][:, ec:ec + n],
                                    start=(f == 0),
                                    stop=(f == FT - 1 and not use_bout))
                            if use_bout:
                                nc.tensor.matmul(
                                    ps[:, :n], ones[0:1, 0:128],
                                    brow[5:6, ec:ec + n], start=False,
                                    stop=True)
                            ch.append((ps[:, :n], ec, n))
                        otile = out_pool.tile([128, E], F32, name="ot",
                                              tag="outp")
                        resid = [hh_t[tt][:, ec:ec + n] for (_, ec, n) in ch]
                        layernorm(ch, otile, 1, use_g2, use_b2, resid=resid)
                        nc.gpsimd.dma_start(
                            d_out[t0 + tt * 128:t0 + (tt + 1) * 128, :],
                            otile[:, :])
    nc.compile()
    return nc


def _get_program(flags):
    key = ("prog", flags)
    if key not in _CACHE:
        _CACHE[key] = _build(flags)
    return _CACHE[key]


def kernel(x, mask, Wq, bq, Wk, bk, Wv, bv, Wo, bo,
           Wso, bso, gso, beso, Wi, bi, Wout, bout, gout, beout):
    from concourse.bass_utils import run_bass_kernel_spmd

    x = np.asarray(x, np.float32)
    mask = np.asarray(mask)
    sc = 1.0 / float(np.sqrt(np.float32(DK)))

    z = lambda a: not np.any(np.asarray(a))
    one = lambda a: bool(np.all(np.asarray(a) == 1.0))
    flags = (not z(bq), not z(bk), not z(bv), not z(bo), not z(bso),
             not z(bi), not z(bout),
             not one(gso), not z(beso), not one(gout), not z(beout))
    nc = _get_program(flags)

    wq_b = _bf(np.asarray(Wq, np.float32) * sc)
    wk_b, wv_b, wo_b, wso_b = _bf(Wk), _bf(Wv), _bf(Wo), _bf(Wso)
    wi_b, wout_b = _bf(Wi), _bf(Wout)
    identb = _bf(np.eye(128))
    onesr = _bf(np.ones((1, 512)))

    brow = np.zeros((7, FF), np.float32)
    brow[0, :E] = np.asarray(bq, np.float32) * sc
    for i, v in enumerate((bk, bv, bo, bso, bout)):
        brow[i + 1, :E] = v
    brow[6, :] = bi
    brow = _bf(brow)
    bicol = np.asarray(bi, np.float32).reshape(FF // 128, 128).T.copy()
    gbt = np.zeros((128, 4 * E), np.float32)
    for i, g in enumerate((gso, gout, beso, beout)):   # gamma1|gamma2|beta1|beta2
        gbt[:, i * E:(i + 1) * E] = np.broadcast_to(
            np.asarray(g, np.float32).reshape(1, E), (128, E))

    in_maps = []
    for c in range(NCORES):
        xs = x[c * BL:(c + 1) * BL].reshape(T, E)
        ms = np.asarray(mask[c * BL:(c + 1) * BL]).reshape(BL, S)
        mbias = _bf(np.where(ms == 0, np.float32(MASK_NEG),
                             np.float32(0.0)).reshape(1, T))
        in_maps.append({
            "x": _bf(xs), "wq": wq_b, "wk": wk_b, "wv": wv_b, "wo": wo_b,
            "wso": wso_b, "wi": wi_b, "wout": wout_b, "mbias": mbias,
            "ident": identb, "onesrow": onesr,
            "brow": brow, "bicol": bicol, "gb": gbt,
        })

    trace = os.environ.get("KERNEL_TRACE", "0") == "1"
    res = run_bass_kernel_spmd(nc, in_maps, core_ids=list(range(NCORES)),
                               trace=trace)
    if trace and res.exec_time_ns is not None:
        print(f"HW exec time: {res.exec_time_ns} ns")
        if res.instructions_and_trace is not None:
            print(f"trace: {res.instructions_and_trace[1]}")
    out = np.concatenate([r["out"].reshape(BL, S, E) for r in res.results],
                         axis=0)
    return np.ascontiguousarray(out.astype(np.float32))


# revision 24
# speedup vs baseline: 1.0144x; 1.0100x over previous
"""BERT-base encoder layer on 8 Trainium2 NeuronCores (Bass/Tile).

Sharding: data-parallel over batch. Full inputs [32, 512, 768] split into 8
shards of 4 batches (2048 tokens); every core runs the same NEFF on its shard
(SPMD, no collectives); host concatenates the outputs.

All GEMMs run on the PE in bf16 with fp32 PSUM accumulation; softmax and
layernorm statistics run in fp32. 1/sqrt(dk) is folded into Wq on the host.
The additive attention mask is applied by a K=1 rank-1 matmul accumulated
into the score PSUM (scores are bounded, so softmax needs no max-subtract).
The softmax normalization (1/rowsum) is folded into the PE transpose of the
probabilities by using diag(1/s) instead of the identity.
"""

import os
import numpy as np
import ml_dtypes

B, S, E, H, DK, FF = 32, 512, 768, 12, 64, 3072
NCORES = 8
BL = B // NCORES          # batches per core = 4
T = BL * S                # tokens per core = 2048
EPS = 1e-12
MASK_NEG = -87.0          # stays inside exp-table range; exp() == 0 in fp32

_CACHE = {}


def _bf(a):
    return np.ascontiguousarray(np.asarray(a, np.float32).astype(ml_dtypes.bfloat16))


def _build(flags):
    import concourse.bass as bass
    import concourse.bacc as bacc
    import concourse.mybir as mybir
    import concourse.tile as tile
    from contextlib import ExitStack

    (use_bq, use_bk, use_bv, use_bo, use_bso, use_bi, use_bout,
     use_g1, use_b1, use_g2, use_b2) = flags

    AF = mybir.ActivationFunctionType
    OP = mybir.AluOpType
    AX = mybir.AxisListType
    BF16 = mybir.dt.bfloat16
    F32 = mybir.dt.float32

    nc = bacc.Bacc("TRN2", target_bir_lowering=False)

    d_x = nc.dram_tensor("x", (T, E), BF16, kind="ExternalInput")
    d_wq = nc.dram_tensor("wq", (E, E), BF16, kind="ExternalInput")
    d_wk = nc.dram_tensor("wk", (E, E), BF16, kind="ExternalInput")
    d_wv = nc.dram_tensor("wv", (E, E), BF16, kind="ExternalInput")
    d_wo = nc.dram_tensor("wo", (E, E), BF16, kind="ExternalInput")
    d_wso = nc.dram_tensor("wso", (E, E), BF16, kind="ExternalInput")
    d_wi = nc.dram_tensor("wi", (E, FF), BF16, kind="ExternalInput")
    d_wout = nc.dram_tensor("wout", (FF, E), BF16, kind="ExternalInput")
    d_mb = nc.dram_tensor("mbias", (1, T), BF16, kind="ExternalInput")
    d_id = nc.dram_tensor("ident", (128, 128), BF16, kind="ExternalInput")
    d_ones = nc.dram_tensor("onesrow", (1, 512), BF16, kind="ExternalInput")
    # bias rows: 0=bq/8, 1=bk, 2=bv, 3=bo, 4=bso, 5=bout, 6=bi (full FF width)
    d_brow = nc.dram_tensor("brow", (7, FF), BF16, kind="ExternalInput")
    d_bic = nc.dram_tensor("bicol", (128, FF // 128), F32, kind="ExternalInput")
    # gamma1 | beta1 | gamma2 | beta2, each [128, 768] partition-broadcast
    d_gb = nc.dram_tensor("gb", (128, 4 * E), F32, kind="ExternalInput")
    d_out = nc.dram_tensor("out", (T, E), F32, kind="ExternalOutput")

    KT_E = E // 128    # 6
    NT_B = S // 128    # 4
    FT = FF // 128     # 24
    HP = H // 2        # 6

    need_gb = use_g1 or use_b1 or use_g2 or use_b2
    need_brow = use_bq or use_bk or use_bv or use_bo or use_bso or use_bout

    with ExitStack() as ctx:
        tc = ctx.enter_context(tile.TileContext(nc))

        p_mm = ctx.enter_context(tc.tile_pool(name="p_mm", bufs=6, space="PSUM"))
        p_sm = ctx.enter_context(tc.tile_pool(name="p_sm", bufs=2, space="PSUM"))

        c_pool = ctx.enter_context(tc.tile_pool(name="consts", bufs=1))
        xt_pool = ctx.enter_context(tc.tile_pool(name="xt", bufs=BL * KT_E))
        att_pool = ctx.enter_context(tc.tile_pool(name="attp", bufs=BL * KT_E + 2))

        ident = c_pool.tile_from(d_id[:, :], name="ident")
        ones = c_pool.tile_from(d_ones[:, :], name="ones")
        brow = c_pool.tile_from(d_brow[:, :], name="brow") if need_brow else None
        gb = c_pool.tile_from(d_gb[:, :], name="gb") if need_gb else None

        XT = {}    # (b, kt) -> [128, S] bf16, feature-major x
        ATT = {}   # (b, kt) -> [128, S] bf16, feature-major attention context

        # ================= superphase A: x^T, QKV, attention =================
        with ExitStack() as sa:
            a_pool = sa.enter_context(tc.tile_pool(name="a_consts", bufs=1))
            wq_pool = sa.enter_context(tc.tile_pool(name="wq", bufs=KT_E))
            wk_pool = sa.enter_context(tc.tile_pool(name="wk", bufs=KT_E))
            wv_pool = sa.enter_context(tc.tile_pool(name="wv", bufs=KT_E))
            xb_pool = sa.enter_context(tc.tile_pool(name="xb", bufs=4))
            qt_pool = sa.enter_context(tc.tile_pool(name="qt", bufs=KT_E + 6))
            kt_pool = sa.enter_context(tc.tile_pool(name="kt", bufs=KT_E + 6))
            v_pool = sa.enter_context(tc.tile_pool(name="v", bufs=NT_B + 2))
            p_pool = sa.enter_context(tc.tile_pool(name="pp", bufs=26))
            pt_pool = sa.enter_context(tc.tile_pool(name="pt", bufs=12))
            dg_pool = sa.enter_context(tc.tile_pool(name="dg", bufs=14))
            s_pool = sa.enter_context(tc.tile_pool(name="sa_s", bufs=16))

            mb = a_pool.tile_from(d_mb[:, :], name="mb")
            WQ = [wq_pool.tile_from(d_wq[k * 128:(k + 1) * 128, :], name="wqt")
                  for k in range(KT_E)]
            WK = [wk_pool.tile_from(d_wk[k * 128:(k + 1) * 128, :], name="wkt")
                  for k in range(KT_E)]
            WV = [wv_pool.tile_from(d_wv[k * 128:(k + 1) * 128, :], name="wvt")
                  for k in range(KT_E)]

            for b in range(BL):
                t0 = b * S
                # ---- x -> XT (feature-major), PE transpose ----
                for kt in range(KT_E):
                    XT[(b, kt)] = xt_pool.tile([128, S], BF16, name="xtt", tag="xt")
                for tt in range(NT_B):
                    xbt = xb_pool.tile([128, E], BF16, name="xbt", tag="xb")
                    nc.gpsimd.dma_start(
                        xbt[:, :], d_x[t0 + tt * 128:t0 + (tt + 1) * 128, :])
                    tps = [p_mm.tile([128, 512], BF16, name="xtp", tag="mm")
                           for _ in range(2)]
                    for et in range(KT_E):
                        sl = tps[et // 4][:, (et % 4) * 128:(et % 4 + 1) * 128]
                        nc.tensor.transpose(
                            sl, xbt[:, et * 128:(et + 1) * 128], ident[:, :])
                    for et in range(KT_E):
                        sl = tps[et // 4][:, (et % 4) * 128:(et % 4 + 1) * 128]
                        nc.vector.tensor_copy(
                            XT[(b, et)][:, tt * 128:(tt + 1) * 128], sl)

                # ---- Q/K projections (feature-major out) ----
                QT, KTt = [None] * KT_E, [None] * KT_E
                V = [None] * NT_B
                for Wt, dstl, pool, ub, brx, tg in (
                        (WQ, QT, qt_pool, use_bq, 0, "qt"),
                        (WK, KTt, kt_pool, use_bk, 1, "kt")):
                    for et in range(KT_E):
                        ps = p_mm.tile([128, S], F32, name="qkps", tag="mm")
                        for k in range(KT_E):
                            nc.tensor.matmul(
                                ps[:, :], Wt[k][:, et * 128:(et + 1) * 128],
                                XT[(b, k)][:, :],
                                start=(k == 0), stop=(k == KT_E - 1 and not ub))
                        if ub:
                            nc.tensor.matmul(
                                ps[:, :],
                                brow[brx:brx + 1, et * 128:(et + 1) * 128],
                                ones[0:1, 0:S], start=False, stop=True)
                        dstl[et] = pool.tile([128, S], BF16, name="qkt", tag=tg)
                        nc.vector.tensor_copy(dstl[et][:, :], ps[:, :])

                # ---- V projection (token-major out) ----
                for tt in range(NT_B):
                    V[tt] = v_pool.tile([128, E], BF16, name="vt", tag="v")
                    for ec, n in ((0, 512), (512, 256)):
                        ps = (p_mm.tile([128, 512], F32, name="vps", tag="mm")
                              if n == 512 else
                              p_sm.tile([128, 256], F32, name="vps2", tag="sm"))
                        for k in range(KT_E):
                            nc.tensor.matmul(
                                ps[:, :n], XT[(b, k)][:, tt * 128:(tt + 1) * 128],
                                WV[k][:, ec:ec + n],
                                start=(k == 0), stop=(k == KT_E - 1 and not use_bv))
                        if use_bv:
                            nc.tensor.matmul(
                                ps[:, :n], ones[0:1, 0:128],
                                brow[2:3, ec:ec + n], start=False, stop=True)
                        nc.vector.tensor_copy(V[tt][:, ec:ec + n], ps[:, :n])

                # ---- attention, per head-pair ----
                for kt in range(KT_E):
                    ATT[(b, kt)] = att_pool.tile([128, S], BF16, name="attt",
                                                 tag="attT")
                # PTn[(hp, hh)][p, jt*512 + q] = P[q, jt*128+p] / s_q
                PTn = {(hp, hh): pt_pool.tile([128, NT_B * 512], BF16,
                                              name="ptn", tag="pt")
                       for hp in range(HP) for hh in range(2)}
                # software-pipelined: group g = all 12 (hp, hh) chains of
                # q-tile g. Emit scores+exp+recip+diag of group g, but the
                # PT matmuls of group g only after group g+1's front half,
                # so the PE queue never blocks on the ACT/DVE round trip.
                front = {}   # it -> list of (hp, hh, pexp, dg)
                def emit_front(it):
                    chains = []
                    for hp in range(HP):
                        for hh in range(2):
                            o = hh * 64
                            ps = p_mm.tile([128, S], F32, name="scps", tag="mm")
                            nc.tensor.matmul(
                                ps[:, :],
                                QT[hp][o:o + 64, it * 128:(it + 1) * 128],
                                KTt[hp][o:o + 64, :], start=True, stop=False)
                            nc.tensor.matmul(
                                ps[:, :], ones[0:1, 0:128], mb[:, t0:t0 + S],
                                start=False, stop=True)
                            pexp = p_pool.tile([128, S], BF16, name="pexp",
                                               tag="p")
                            ssum = s_pool.tile([128, 1], F32, name="ssum",
                                               tag="ss", bufs=64)
                            nc.scalar.activation(pexp[:, :], ps[:, :], AF.Exp,
                                                 accum_out=ssum[:, :])
                            rs = s_pool.tile([128, 1], F32, name="rsum",
                                             tag="rs")
                            nc.vector.reciprocal(rs[:, :], ssum[:, :])
                            dg = dg_pool.tile([128, 128], BF16, name="dgt",
                                              tag="dg")
                            nc.vector.tensor_scalar_mul(dg[:, :], ident[:, :],
                                                        rs[:, :])
                            chains.append((hp, hh, pexp, dg))
                    front[it] = chains

                def emit_back(it):
                    for hp, hh, pexp, dg in front.pop(it):
                        ptp = p_mm.tile([128, 512], F32, name="ptp", tag="mm")
                        for jc in range(NT_B):
                            nc.tensor.matmul(
                                ptp[:, jc * 128:(jc + 1) * 128],
                                pexp[:, jc * 128:(jc + 1) * 128], dg[:, :],
                                start=True, stop=True)
                        srcv = ptp.rearrange("p (j c) -> p j c", j=NT_B)
                        dst = PTn[(hp, hh)].rearrange("p (j c) -> p j c",
                                                      j=NT_B)
                        nc.vector.tensor_copy(
                            dst[:, :, it * 128:(it + 1) * 128], srcv)

                emit_front(0)
                for it in range(1, NT_B):
                    emit_front(it)
                    emit_back(it - 1)
                emit_back(NT_B - 1)
                for hp in range(HP):
                    ps = p_mm.tile([128, S], F32, name="avps", tag="mm")
                    for hh in range(2):
                        o = hh * 64
                        for jt in range(NT_B):
                            nc.tensor.matmul(
                                ps[o:o + 64, :],
                                V[jt][:, hp * 128 + o:hp * 128 + o + 64],
                                PTn[(hp, hh)][:, jt * 512:(jt + 1) * 512],
                                start=(jt == 0), stop=(jt == NT_B - 1),
                                tile_position=(0, o))
                    nc.vector.tensor_copy(ATT[(b, hp)][:, :], ps[:, :])

        # ============ superphase B: O-proj, SelfOutput LN, FFN, LN ===========
        with ExitStack() as sb:
            b_pool = sb.enter_context(tc.tile_pool(name="b_consts", bufs=1))
            wo_pool = sb.enter_context(tc.tile_pool(name="wo", bufs=KT_E))
            wso_pool = sb.enter_context(tc.tile_pool(name="wso", bufs=KT_E))
            wi_pool = sb.enter_context(tc.tile_pool(name="wi", bufs=KT_E))
            wout_pool = sb.enter_context(tc.tile_pool(name="wout", bufs=FT))
            h_pool = sb.enter_context(tc.tile_pool(name="h", bufs=NT_B + 1))
            ht_pool = sb.enter_context(tc.tile_pool(name="ht", bufs=2))
            fft_pool = sb.enter_context(tc.tile_pool(name="fft", bufs=FT + 2))
            sq_pool = sb.enter_context(tc.tile_pool(name="sq", bufs=2))
            rs_pool = sb.enter_context(tc.tile_pool(name="rsd", bufs=2))
            out_pool = sb.enter_context(tc.tile_pool(name="outp", bufs=2))
            t_pool = sb.enter_context(tc.tile_pool(name="sb_s", bufs=12))

            bic = b_pool.tile_from(d_bic[:, :], name="bic") if use_bi else None
            WO = [wo_pool.tile_from(d_wo[k * 128:(k + 1) * 128, :], name="wot")
                  for k in range(KT_E)]
            WSO = [wso_pool.tile_from(d_wso[k * 128:(k + 1) * 128, :], name="wsot")
                   for k in range(KT_E)]
            WI = [wi_pool.tile_from(d_wi[k * 128:(k + 1) * 128, :], name="wit")
                  for k in range(KT_E)]
            WOUT = [wout_pool.tile_from(d_wout[f * 128:(f + 1) * 128, :],
                                        name="woutt") for f in range(FT)]

            def layernorm(chunks, h_dst, gcol, use_g, use_bb, resid=None):
                """chunks: [(psum_ap, col0, n)]; h_dst: [128, E] fp32 out.
                resid: parallel list of sbuf fp32 APs added to psum first."""
                if resid is not None:
                    rtile = rs_pool.tile([128, E], F32, name="rt", tag="rsd")
                    for (ps, c0, n), rext in zip(chunks, resid):
                        nc.vector.scalar_tensor_tensor(
                            rtile[:, c0:c0 + n], ps, 1.0, rext,
                            op0=OP.mult, op1=OP.add)
                    srcs = [(rtile[:, c0:c0 + n], c0, n) for (_, c0, n) in chunks]
                else:
                    srcs = chunks
                s1 = t_pool.tile([128, 1], F32, name="s1", tag="s1")
                s1b = t_pool.tile([128, 1], F32, name="s1b", tag="s1b")
                nc.vector.reduce_sum(s1[:, :], srcs[0][0], axis=AX.X)
                nc.vector.reduce_sum(s1b[:, :], srcs[1][0], axis=AX.X)
                mu_n = t_pool.tile([128, 1], F32, name="mun", tag="mun")
                # mu_n = -(s1 + s1b)/E
                tmp = t_pool.tile([128, 1], F32, name="tmps", tag="tmps")
                nc.vector.scalar_tensor_tensor(
                    tmp[:, :], s1[:, :], 1.0, s1b[:, :], op0=OP.mult, op1=OP.add)
                nc.vector.tensor_scalar_mul(mu_n[:, :], tmp[:, :], -1.0 / E)
                ss = t_pool.tile([128, 1], F32, name="ssa", tag="ssa", bufs=34)
                ssb = t_pool.tile([128, 1], F32, name="ssb", tag="ssb", bufs=34)
                for (src, c0, n), acc in zip(srcs, (ss, ssb)):
                    sq = sq_pool.tile([128, 512], BF16, name="sqt", tag="sq")
                    nc.scalar.activation(sq[:, :n], src, AF.Square,
                                         accum_out=acc[:, :])
                # var = (ss+ssb)/E - mu^2 ; rstd = 1/sqrt(var + eps)
                musq = t_pool.tile([128, 1], F32, name="musq", tag="musq")
                nc.vector.scalar_tensor_tensor(
                    musq[:, :], mu_n[:, :], 1.0, mu_n[:, :],
                    op0=OP.mult, op1=OP.mult)
                veps = t_pool.tile([128, 1], F32, name="veps", tag="veps")
                nc.vector.scalar_tensor_tensor(
                    veps[:, :], ss[:, :], 1.0, ssb[:, :],
                    op0=OP.mult, op1=OP.add)
                veps2 = t_pool.tile([128, 1], F32, name="veps2", tag="veps2")
                nc.vector.tensor_scalar(
                    veps2[:, :], veps[:, :], 1.0 / E, EPS,
                    op0=OP.mult, op1=OP.add)
                veps3 = t_pool.tile([128, 1], F32, name="veps3", tag="veps3")
                nc.vector.scalar_tensor_tensor(
                    veps3[:, :], musq[:, :], -1.0, veps2[:, :],
                    op0=OP.mult, op1=OP.add)
                sd = t_pool.tile([128, 1], F32, name="sd", tag="sd")
                nc.scalar.sqrt(sd[:, :], veps3[:, :])
                rstd = t_pool.tile([128, 1], F32, name="rstd", tag="rstd")
                nc.vector.reciprocal(rstd[:, :], sd[:, :])
                for (src, c0, n) in srcs:
                    nc.vector.tensor_scalar(
                        h_dst[:, c0:c0 + n], src, mu_n[:, :], rstd[:, :],
                        op0=OP.add, op1=OP.mult)
                if use_g:
                    nc.vector.scalar_tensor_tensor(
                        h_dst[:, :], h_dst[:, :], 1.0,
                        gb[:, gcol * E:(gcol + 1) * E], op0=OP.mult, op1=OP.mult)
                if use_bb:
                    nc.vector.scalar_tensor_tensor(
                        h_dst[:, :], h_dst[:, :], 1.0,
                        gb[:, (gcol + 2) * E:(gcol + 3) * E],
                        op0=OP.mult, op1=OP.add)

            for b in range(BL):
                t0 = b * S
                # ---- O-projection + residual -> xa (feature-major bf16) ----
                xa = [None] * KT_E
                for et in range(KT_E):
                    ps = p_mm.tile([128, S], F32, name="ops", tag="mm")
                    for k in range(KT_E):
                        nc.tensor.matmul(
                            ps[:, :], WO[k][:, et * 128:(et + 1) * 128],
                            ATT[(b, k)][:, :],
                            start=(k == 0), stop=(k == KT_E - 1 and not use_bo))
                    if use_bo:
                        nc.tensor.matmul(
                            ps[:, :], brow[3:4, et * 128:(et + 1) * 128],
                            ones[0:1, 0:S], start=False, stop=True)
                    xa[et] = att_pool.tile([128, S], BF16, name="xat", tag="attT")
                    nc.vector.scalar_tensor_tensor(
                        xa[et][:, :], ps[:, :], 1.0, XT[(b, et)][:, :],
                        op0=OP.mult, op1=OP.add)

                # ---- SelfOutput GEMM + LN1 -> h (token-major fp32), hT ----
                hh_t = [None] * NT_B
                hT = ht_pool.tile([128, KT_E * S], BF16, name="htt", tag="ht")

                def emit_htrans(tt):
                    tps = [p_mm.tile([128, 512], BF16, name="htp", tag="mm")
                           for _ in range(2)]
                    for et in range(KT_E):
                        sl = tps[et // 4][:, (et % 4) * 128:(et % 4 + 1) * 128]
                        nc.tensor.transpose(
                            sl, hh_t[tt][:, et * 128:(et + 1) * 128],
                            ident[:, :])
                    for et in range(KT_E):
                        sl = tps[et // 4][:, (et % 4) * 128:(et % 4 + 1) * 128]
                        nc.vector.tensor_copy(
                            hT[:, et * S + tt * 128:et * S + (tt + 1) * 128], sl)

                # skewed: h-transposes of tile tt are emitted after the
                # SO GEMM of tile tt+1, so the PE never waits on LN1
                for tt in range(NT_B):
                    ch = []
                    for ec, n in ((0, 512), (512, 256)):
                        ps = (p_mm.tile([128, 512], F32, name="sops", tag="mm")
                              if n == 512 else
                              p_mm.tile([128, 256], F32, name="sops2", tag="mm"))
                        for k in range(KT_E):
                            nc.tensor.matmul(
                                ps[:, :n], xa[k][:, tt * 128:(tt + 1) * 128],
                                WSO[k][:, ec:ec + n],
                                start=(k == 0),
                                stop=(k == KT_E - 1 and not use_bso))
                        if use_bso:
                            nc.tensor.matmul(
                                ps[:, :n], ones[0:1, 0:128],
                                brow[4:5, ec:ec + n], start=False, stop=True)
                        ch.append((ps[:, :n], ec, n))
                    hh_t[tt] = h_pool.tile([128, E], BF16, name="hht", tag="h")
                    layernorm(ch, hh_t[tt], 0, use_g1, use_b1)
                    if tt > 0:
                        emit_htrans(tt - 1)
                emit_htrans(NT_B - 1)

                # ---- FFN + LN2 (full 512-token chunk: N=512 Wi GEMMs) ----
                ffT = [None] * FT
                for ft in range(FT):
                    ps = p_mm.tile([128, 512], F32, name="fips", tag="mm")
                    for k in range(KT_E):
                        nc.tensor.matmul(
                            ps[:, :], WI[k][:, ft * 128:(ft + 1) * 128],
                            hT[:, k * S:k * S + 512],
                            start=(k == 0), stop=(k == KT_E - 1))
                    ffT[ft] = fft_pool.tile([128, 512], BF16, name="fftt",
                                            tag="fft")
                    if use_bi:
                        nc.scalar.activation(ffT[ft][:, :], ps[:, :],
                                             AF.Gelu,
                                             bias=bic[:, ft:ft + 1])
                    else:
                        nc.scalar.activation(ffT[ft][:, :], ps[:, :],
                                             AF.Gelu)
                for tt in range(NT_B):
                    ch = []
                    for ec, n in ((0, 512), (512, 256)):
                        ps = (p_mm.tile([128, 512], F32, name="wops",
                                        tag="mm") if n == 512 else
                              p_mm.tile([128, 256], F32, name="wops2",
                                        tag="mm"))
                        for f in range(FT):
                            nc.tensor.matmul(
                                ps[:, :n],
                                ffT[f][:, tt * 128:(tt + 1) * 128],
                                WOUT[f][:, ec:ec + n],
                                start=(f == 0),
                                stop=(f == FT - 1 and not use_bout))
                        if use_bout:
                            nc.tensor.matmul(
                                ps[:, :n], ones[0:1, 0:128],
                                brow[5:6, ec:ec + n], start=False,
                                stop=True)
                        ch.append((ps[:, :n], ec, n))
                    otile = out_pool.tile([128, E], F32, name="ot",
                                          tag="outp")
                    resid = [hh_t[tt][:, ec:ec + n] for (_, ec, n) in ch]
                    layernorm(ch, otile, 1, use_g2, use_b2, resid=resid)
                    nc.gpsimd.dma_start(
                        d_out[t0 + tt * 128:t0 + (tt + 1) * 128, :],
                        otile[:, :])
    nc.compile()
    return nc


def _get_program(flags):
    key = ("prog", flags)
    if key not in _CACHE:
        _CACHE[key] = _build(flags)
    return _CACHE[key]


def kernel(x, mask, Wq, bq, Wk, bk, Wv, bv, Wo, bo,
           Wso, bso, gso, beso, Wi, bi, Wout, bout, gout, beout):
    from concourse.bass_utils import run_bass_kernel_spmd

    x = np.asarray(x, np.float32)
    mask = np.asarray(mask)
    sc = 1.0 / float(np.sqrt(np.float32(DK)))

    z = lambda a: not np.any(np.asarray(a))
    one = lambda a: bool(np.all(np.asarray(a) == 1.0))
    flags = (not z(bq), not z(bk), not z(bv), not z(bo), not z(bso),
             not z(bi), not z(bout),
             not one(gso), not z(beso), not one(gout), not z(beout))
    nc = _get_program(flags)

    wq_b = _bf(np.asarray(Wq, np.float32) * sc)
    wk_b, wv_b, wo_b, wso_b = _bf(Wk), _bf(Wv), _bf(Wo), _bf(Wso)
    wi_b, wout_b = _bf(Wi), _bf(Wout)
    identb = _bf(np.eye(128))
    onesr = _bf(np.ones((1, 512)))

    brow = np.zeros((7, FF), np.float32)
    brow[0, :E] = np.asarray(bq, np.float32) * sc
    for i, v in enumerate((bk, bv, bo, bso, bout)):
        brow[i + 1, :E] = v
    brow[6, :] = bi
    brow = _bf(brow)
    bicol = np.asarray(bi, np.float32).reshape(FF // 128, 128).T.copy()
    gbt = np.zeros((128, 4 * E), np.float32)
    for i, g in enumerate((gso, gout, beso, beout)):   # gamma1|gamma2|beta1|beta2
        gbt[:, i * E:(i + 1) * E] = np.broadcast_to(
            np.asarray(g, np.float32).reshape(1, E), (128, E))

    in_maps = []
    for c in range(NCORES):
        xs = x[c * BL:(c + 1) * BL].reshape(T, E)
        ms = np.asarray(mask[c * BL:(c + 1) * BL]).reshape(BL, S)
        mbias = _bf(np.where(ms == 0, np.float32(MASK_NEG),
                             np.float32(0.0)).reshape(1, T))
        in_maps.append({
            "x": _bf(xs), "wq": wq_b, "wk": wk_b, "wv": wv_b, "wo": wo_b,
            "wso": wso_b, "wi": wi_b, "wout": wout_b, "mbias": mbias,
            "ident": identb, "onesrow": onesr,
            "brow": brow, "bicol": bicol, "gb": gbt,
        })

    trace = os.environ.get("KERNEL_TRACE", "0") == "1"
    res = run_bass_kernel_spmd(nc, in_maps, core_ids=list(range(NCORES)),
                               trace=trace)
    if trace and res.exec_time_ns is not None:
        print(f"HW exec time: {res.exec_time_ns} ns")
        if res.instructions_and_trace is not None:
            print(f"trace: {res.instructions_and_trace[1]}")
    out = np.concatenate([r["out"].reshape(BL, S, E) for r in res.results],
                         axis=0)
    return np.ascontiguousarray(out.astype(np.float32))


# revision 25
# speedup vs baseline: 1.0340x; 1.0193x over previous
"""BERT-base encoder layer on 8 Trainium2 NeuronCores (Bass/Tile).

Sharding: data-parallel over batch. Full inputs [32, 512, 768] split into 8
shards of 4 batches (2048 tokens); every core runs the same NEFF on its shard
(SPMD, no collectives); host concatenates the outputs.

All GEMMs run on the PE in bf16 with fp32 PSUM accumulation; softmax and
layernorm statistics run in fp32. 1/sqrt(dk) is folded into Wq on the host.
The additive attention mask is applied by a K=1 rank-1 matmul accumulated
into the score PSUM (scores are bounded, so softmax needs no max-subtract).
The softmax normalization (1/rowsum) is folded into the PE transpose of the
probabilities by using diag(1/s) instead of the identity.
"""

import os
import numpy as np
import ml_dtypes

B, S, E, H, DK, FF = 32, 512, 768, 12, 64, 3072
NCORES = 8
BL = B // NCORES          # batches per core = 4
T = BL * S                # tokens per core = 2048
EPS = 1e-12
MASK_NEG = -87.0          # stays inside exp-table range; exp() == 0 in fp32

_CACHE = {}


def _bf(a):
    return np.ascontiguousarray(np.asarray(a, np.float32).astype(ml_dtypes.bfloat16))


def _build(flags):
    import concourse.bass as bass
    import concourse.bacc as bacc
    import concourse.mybir as mybir
    import concourse.tile as tile
    from contextlib import ExitStack

    (use_bq, use_bk, use_bv, use_bo, use_bso, use_bi, use_bout,
     use_g1, use_b1, use_g2, use_b2) = flags

    AF = mybir.ActivationFunctionType
    OP = mybir.AluOpType
    AX = mybir.AxisListType
    BF16 = mybir.dt.bfloat16
    F32 = mybir.dt.float32

    nc = bacc.Bacc("TRN2", target_bir_lowering=False)

    d_x = nc.dram_tensor("x", (T, E), BF16, kind="ExternalInput")
    d_wq = nc.dram_tensor("wq", (E, E), BF16, kind="ExternalInput")
    d_wk = nc.dram_tensor("wk", (E, E), BF16, kind="ExternalInput")
    d_wv = nc.dram_tensor("wv", (E, E), BF16, kind="ExternalInput")
    d_wo = nc.dram_tensor("wo", (E, E), BF16, kind="ExternalInput")
    d_wso = nc.dram_tensor("wso", (E, E), BF16, kind="ExternalInput")
    d_wi = nc.dram_tensor("wi", (E, FF), BF16, kind="ExternalInput")
    d_wout = nc.dram_tensor("wout", (FF, E), BF16, kind="ExternalInput")
    d_mb = nc.dram_tensor("mbias", (1, T), BF16, kind="ExternalInput")
    d_id = nc.dram_tensor("ident", (128, 128), BF16, kind="ExternalInput")
    d_ones = nc.dram_tensor("onesrow", (1, 512), BF16, kind="ExternalInput")
    # bias rows: 0=bq/8, 1=bk, 2=bv, 3=bo, 4=bso, 5=bout, 6=bi (full FF width)
    d_brow = nc.dram_tensor("brow", (7, FF), BF16, kind="ExternalInput")
    d_bic = nc.dram_tensor("bicol", (128, FF // 128), F32, kind="ExternalInput")
    # gamma1 | beta1 | gamma2 | beta2, each [128, 768] partition-broadcast
    d_gb = nc.dram_tensor("gb", (128, 4 * E), F32, kind="ExternalInput")
    d_out = nc.dram_tensor("out", (T, E), F32, kind="ExternalOutput")

    KT_E = E // 128    # 6
    NT_B = S // 128    # 4
    FT = FF // 128     # 24
    HP = H // 2        # 6

    need_gb = use_g1 or use_b1 or use_g2 or use_b2
    need_brow = use_bq or use_bk or use_bv or use_bo or use_bso or use_bout

    with ExitStack() as ctx:
        tc = ctx.enter_context(tile.TileContext(nc))

        p_mm = ctx.enter_context(tc.tile_pool(name="p_mm", bufs=8, space="PSUM"))

        c_pool = ctx.enter_context(tc.tile_pool(name="consts", bufs=1))
        xt_pool = ctx.enter_context(tc.tile_pool(name="xt", bufs=BL * KT_E))
        att_pool = ctx.enter_context(tc.tile_pool(name="attp", bufs=BL * KT_E + 2))

        ident = c_pool.tile_from(d_id[:, :], name="ident")
        ones = c_pool.tile_from(d_ones[:, :], name="ones")
        brow = c_pool.tile_from(d_brow[:, :], name="brow") if need_brow else None
        gb = c_pool.tile_from(d_gb[:, :], name="gb") if need_gb else None

        XT = {}    # (b, kt) -> [128, S] bf16, feature-major x
        ATT = {}   # (b, kt) -> [128, S] bf16, feature-major attention context

        # ================= superphase A: x^T, QKV, attention =================
        with ExitStack() as sa:
            a_pool = sa.enter_context(tc.tile_pool(name="a_consts", bufs=1))
            wq_pool = sa.enter_context(tc.tile_pool(name="wq", bufs=KT_E))
            wk_pool = sa.enter_context(tc.tile_pool(name="wk", bufs=KT_E))
            wv_pool = sa.enter_context(tc.tile_pool(name="wv", bufs=KT_E))
            xb_pool = sa.enter_context(tc.tile_pool(name="xb", bufs=4))
            qt_pool = sa.enter_context(tc.tile_pool(name="qt", bufs=KT_E + 6))
            kt_pool = sa.enter_context(tc.tile_pool(name="kt", bufs=KT_E + 6))
            v_pool = sa.enter_context(tc.tile_pool(name="v", bufs=NT_B + 2))
            p_pool = sa.enter_context(tc.tile_pool(name="pp", bufs=26))
            pt_pool = sa.enter_context(tc.tile_pool(name="pt", bufs=12))
            dg_pool = sa.enter_context(tc.tile_pool(name="dg", bufs=14))
            s_pool = sa.enter_context(tc.tile_pool(name="sa_s", bufs=16))

            mb = a_pool.tile_from(d_mb[:, :], name="mb")
            WQ = [wq_pool.tile_from(d_wq[k * 128:(k + 1) * 128, :], name="wqt")
                  for k in range(KT_E)]
            WK = [wk_pool.tile_from(d_wk[k * 128:(k + 1) * 128, :], name="wkt")
                  for k in range(KT_E)]
            WV = [wv_pool.tile_from(d_wv[k * 128:(k + 1) * 128, :], name="wvt")
                  for k in range(KT_E)]

            for b in range(BL):
                t0 = b * S
                # ---- x -> XT (feature-major), PE transpose ----
                for kt in range(KT_E):
                    XT[(b, kt)] = xt_pool.tile([128, S], BF16, name="xtt", tag="xt")
                for tt in range(NT_B):
                    xbt = xb_pool.tile([128, E], BF16, name="xbt", tag="xb")
                    nc.gpsimd.dma_start(
                        xbt[:, :], d_x[t0 + tt * 128:t0 + (tt + 1) * 128, :])
                    tps = [p_mm.tile([128, 512], BF16, name="xtp", tag="mm")
                           for _ in range(2)]
                    for et in range(KT_E):
                        sl = tps[et // 4][:, (et % 4) * 128:(et % 4 + 1) * 128]
                        nc.tensor.transpose(
                            sl, xbt[:, et * 128:(et + 1) * 128], ident[:, :])
                    for et in range(KT_E):
                        sl = tps[et // 4][:, (et % 4) * 128:(et % 4 + 1) * 128]
                        nc.vector.tensor_copy(
                            XT[(b, et)][:, tt * 128:(tt + 1) * 128], sl)

                # ---- Q/K projections (feature-major out) ----
                QT, KTt = [None] * KT_E, [None] * KT_E
                V = [None] * NT_B
                for Wt, dstl, pool, ub, brx, tg in (
                        (WQ, QT, qt_pool, use_bq, 0, "qt"),
                        (WK, KTt, kt_pool, use_bk, 1, "kt")):
                    for et in range(KT_E):
                        ps = p_mm.tile([128, S], F32, name="qkps", tag="mm")
                        for k in range(KT_E):
                            nc.tensor.matmul(
                                ps[:, :], Wt[k][:, et * 128:(et + 1) * 128],
                                XT[(b, k)][:, :],
                                start=(k == 0), stop=(k == KT_E - 1 and not ub))
                        if ub:
                            nc.tensor.matmul(
                                ps[:, :],
                                brow[brx:brx + 1, et * 128:(et + 1) * 128],
                                ones[0:1, 0:S], start=False, stop=True)
                        dstl[et] = pool.tile([128, S], BF16, name="qkt", tag=tg)
                        nc.vector.tensor_copy(dstl[et][:, :], ps[:, :])

                # ---- V projection (token-major out) ----
                for tt in range(NT_B):
                    V[tt] = v_pool.tile([128, E], BF16, name="vt", tag="v")
                    for ec, n in ((0, 512), (512, 256)):
                        ps = (p_mm.tile([128, 512], F32, name="vps", tag="mm")
                              if n == 512 else
                              p_mm.tile([128, 256], F32, name="vps2", tag="mm"))
                        for k in range(KT_E):
                            nc.tensor.matmul(
                                ps[:, :n], XT[(b, k)][:, tt * 128:(tt + 1) * 128],
                                WV[k][:, ec:ec + n],
                                start=(k == 0), stop=(k == KT_E - 1 and not use_bv))
                        if use_bv:
                            nc.tensor.matmul(
                                ps[:, :n], ones[0:1, 0:128],
                                brow[2:3, ec:ec + n], start=False, stop=True)
                        nc.vector.tensor_copy(V[tt][:, ec:ec + n], ps[:, :n])

                # ---- attention, per head-pair ----
                for kt in range(KT_E):
                    ATT[(b, kt)] = att_pool.tile([128, S], BF16, name="attt",
                                                 tag="attT")
                # PTn[(hp, hh)][p, jt*512 + q] = P[q, jt*128+p] / s_q
                PTn = {(hp, hh): pt_pool.tile([128, NT_B * 512], BF16,
                                              name="ptn", tag="pt")
                       for hp in range(HP) for hh in range(2)}
                # software-pipelined: group g = all 12 (hp, hh) chains of
                # q-tile g. Emit scores+exp+recip+diag of group g, but the
                # PT matmuls of group g only after group g+1's front half,
                # so the PE queue never blocks on the ACT/DVE round trip.
                front = {}   # it -> list of (hp, hh, pexp, dg)
                def emit_front(it):
                    chains = []
                    for hp in range(HP):
                        for hh in range(2):
                            o = hh * 64
                            ps = p_mm.tile([128, S], F32, name="scps", tag="mm")
                            nc.tensor.matmul(
                                ps[:, :],
                                QT[hp][o:o + 64, it * 128:(it + 1) * 128],
                                KTt[hp][o:o + 64, :], start=True, stop=False)
                            nc.tensor.matmul(
                                ps[:, :], ones[0:1, 0:128], mb[:, t0:t0 + S],
                                start=False, stop=True)
                            pexp = p_pool.tile([128, S], BF16, name="pexp",
                                               tag="p")
                            ssum = s_pool.tile([128, 1], F32, name="ssum",
                                               tag="ss", bufs=64)
                            nc.scalar.activation(pexp[:, :], ps[:, :], AF.Exp,
                                                 accum_out=ssum[:, :])
                            rs = s_pool.tile([128, 1], F32, name="rsum",
                                             tag="rs")
                            nc.vector.reciprocal(rs[:, :], ssum[:, :])
                            dg = dg_pool.tile([128, 128], BF16, name="dgt",
                                              tag="dg")
                            nc.vector.tensor_scalar_mul(dg[:, :], ident[:, :],
                                                        rs[:, :])
                            chains.append((hp, hh, pexp, dg))
                    front[it] = chains

                def emit_back(it):
                    for hp, hh, pexp, dg in front.pop(it):
                        ptp = p_mm.tile([128, 512], F32, name="ptp", tag="mm")
                        for jc in range(NT_B):
                            nc.tensor.matmul(
                                ptp[:, jc * 128:(jc + 1) * 128],
                                pexp[:, jc * 128:(jc + 1) * 128], dg[:, :],
                                start=True, stop=True)
                        srcv = ptp.rearrange("p (j c) -> p j c", j=NT_B)
                        dst = PTn[(hp, hh)].rearrange("p (j c) -> p j c",
                                                      j=NT_B)
                        nc.vector.tensor_copy(
                            dst[:, :, it * 128:(it + 1) * 128], srcv)

                emit_front(0)
                for it in range(1, NT_B):
                    emit_front(it)
                    emit_back(it - 1)
                emit_back(NT_B - 1)
                for hp in range(HP):
                    ps = p_mm.tile([128, S], F32, name="avps", tag="mm")
                    for hh in range(2):
                        o = hh * 64
                        for jt in range(NT_B):
                            nc.tensor.matmul(
                                ps[o:o + 64, :],
                                V[jt][:, hp * 128 + o:hp * 128 + o + 64],
                                PTn[(hp, hh)][:, jt * 512:(jt + 1) * 512],
                                start=(jt == 0), stop=(jt == NT_B - 1),
                                tile_position=(0, o))
                    nc.vector.tensor_copy(ATT[(b, hp)][:, :], ps[:, :])

        # ============ superphase B: O-proj, SelfOutput LN, FFN, LN ===========
        with ExitStack() as sb:
            b_pool = sb.enter_context(tc.tile_pool(name="b_consts", bufs=1))
            wo_pool = sb.enter_context(tc.tile_pool(name="wo", bufs=KT_E))
            wso_pool = sb.enter_context(tc.tile_pool(name="wso", bufs=KT_E))
            wi_pool = sb.enter_context(tc.tile_pool(name="wi", bufs=KT_E))
            wout_pool = sb.enter_context(tc.tile_pool(name="wout", bufs=FT))
            h_pool = sb.enter_context(tc.tile_pool(name="h", bufs=NT_B + 1))
            ht_pool = sb.enter_context(tc.tile_pool(name="ht", bufs=2))
            fft_pool = sb.enter_context(tc.tile_pool(name="fft", bufs=FT + 2))
            sq_pool = sb.enter_context(tc.tile_pool(name="sq", bufs=2))
            rs_pool = sb.enter_context(tc.tile_pool(name="rsd", bufs=2))
            out_pool = sb.enter_context(tc.tile_pool(name="outp", bufs=2))
            t_pool = sb.enter_context(tc.tile_pool(name="sb_s", bufs=12))

            bic = b_pool.tile_from(d_bic[:, :], name="bic") if use_bi else None
            WO = [wo_pool.tile_from(d_wo[k * 128:(k + 1) * 128, :], name="wot")
                  for k in range(KT_E)]
            WSO = [wso_pool.tile_from(d_wso[k * 128:(k + 1) * 128, :], name="wsot")
                   for k in range(KT_E)]
            WI = [wi_pool.tile_from(d_wi[k * 128:(k + 1) * 128, :], name="wit")
                  for k in range(KT_E)]
            WOUT = [wout_pool.tile_from(d_wout[f * 128:(f + 1) * 128, :],
                                        name="woutt") for f in range(FT)]

            def layernorm(chunks, h_dst, gcol, use_g, use_bb, resid=None):
                """chunks: [(psum_ap, col0, n)]; h_dst: [128, E] fp32 out.
                resid: parallel list of sbuf fp32 APs added to psum first."""
                if resid is not None:
                    rtile = rs_pool.tile([128, E], F32, name="rt", tag="rsd")
                    for (ps, c0, n), rext in zip(chunks, resid):
                        nc.vector.scalar_tensor_tensor(
                            rtile[:, c0:c0 + n], ps, 1.0, rext,
                            op0=OP.mult, op1=OP.add)
                    srcs = [(rtile[:, c0:c0 + n], c0, n) for (_, c0, n) in chunks]
                else:
                    srcs = chunks
                s1 = t_pool.tile([128, 1], F32, name="s1", tag="s1")
                s1b = t_pool.tile([128, 1], F32, name="s1b", tag="s1b")
                nc.vector.reduce_sum(s1[:, :], srcs[0][0], axis=AX.X)
                nc.vector.reduce_sum(s1b[:, :], srcs[1][0], axis=AX.X)
                mu_n = t_pool.tile([128, 1], F32, name="mun", tag="mun")
                # mu_n = -(s1 + s1b)/E
                tmp = t_pool.tile([128, 1], F32, name="tmps", tag="tmps")
                nc.vector.scalar_tensor_tensor(
                    tmp[:, :], s1[:, :], 1.0, s1b[:, :], op0=OP.mult, op1=OP.add)
                nc.vector.tensor_scalar_mul(mu_n[:, :], tmp[:, :], -1.0 / E)
                ss = t_pool.tile([128, 1], F32, name="ssa", tag="ssa", bufs=34)
                ssb = t_pool.tile([128, 1], F32, name="ssb", tag="ssb", bufs=34)
                for (src, c0, n), acc in zip(srcs, (ss, ssb)):
                    sq = sq_pool.tile([128, 512], BF16, name="sqt", tag="sq")
                    nc.scalar.activation(sq[:, :n], src, AF.Square,
                                         accum_out=acc[:, :])
                # var = (ss+ssb)/E - mu^2 ; rstd = 1/sqrt(var + eps)
                musq = t_pool.tile([128, 1], F32, name="musq", tag="musq")
                nc.vector.scalar_tensor_tensor(
                    musq[:, :], mu_n[:, :], 1.0, mu_n[:, :],
                    op0=OP.mult, op1=OP.mult)
                veps = t_pool.tile([128, 1], F32, name="veps", tag="veps")
                nc.vector.scalar_tensor_tensor(
                    veps[:, :], ss[:, :], 1.0, ssb[:, :],
                    op0=OP.mult, op1=OP.add)
                veps2 = t_pool.tile([128, 1], F32, name="veps2", tag="veps2")
                nc.vector.tensor_scalar(
                    veps2[:, :], veps[:, :], 1.0 / E, EPS,
                    op0=OP.mult, op1=OP.add)
                veps3 = t_pool.tile([128, 1], F32, name="veps3", tag="veps3")
                nc.vector.scalar_tensor_tensor(
                    veps3[:, :], musq[:, :], -1.0, veps2[:, :],
                    op0=OP.mult, op1=OP.add)
                sd = t_pool.tile([128, 1], F32, name="sd", tag="sd")
                nc.scalar.sqrt(sd[:, :], veps3[:, :])
                rstd = t_pool.tile([128, 1], F32, name="rstd", tag="rstd")
                nc.vector.reciprocal(rstd[:, :], sd[:, :])
                for (src, c0, n) in srcs:
                    nc.vector.tensor_scalar(
                        h_dst[:, c0:c0 + n], src, mu_n[:, :], rstd[:, :],
                        op0=OP.add, op1=OP.mult)
                if use_g:
                    nc.vector.scalar_tensor_tensor(
                        h_dst[:, :], h_dst[:, :], 1.0,
                        gb[:, gcol * E:(gcol + 1) * E], op0=OP.mult, op1=OP.mult)
                if use_bb:
                    nc.vector.scalar_tensor_tensor(
                        h_dst[:, :], h_dst[:, :], 1.0,
                        gb[:, (gcol + 2) * E:(gcol + 3) * E],
                        op0=OP.mult, op1=OP.add)

            for b in range(BL):
                t0 = b * S
                # ---- O-projection + residual -> xa (feature-major bf16) ----
                xa = [None] * KT_E
                for et in range(KT_E):
                    ps = p_mm.tile([128, S], F32, name="ops", tag="mm")
                    for k in range(KT_E):
                        nc.tensor.matmul(
                            ps[:, :], WO[k][:, et * 128:(et + 1) * 128],
                            ATT[(b, k)][:, :],
                            start=(k == 0), stop=(k == KT_E - 1 and not use_bo))
                    if use_bo:
                        nc.tensor.matmul(
                            ps[:, :], brow[3:4, et * 128:(et + 1) * 128],
                            ones[0:1, 0:S], start=False, stop=True)
                    xa[et] = att_pool.tile([128, S], BF16, name="xat", tag="attT")
                    nc.vector.scalar_tensor_tensor(
                        xa[et][:, :], ps[:, :], 1.0, XT[(b, et)][:, :],
                        op0=OP.mult, op1=OP.add)

                # ---- SelfOutput GEMM + LN1 -> h (token-major fp32), hT ----
                hh_t = [None] * NT_B
                hT = ht_pool.tile([128, KT_E * S], BF16, name="htt", tag="ht")

                def emit_htrans(tt):
                    tps = [p_mm.tile([128, 512], BF16, name="htp", tag="mm")
                           for _ in range(2)]
                    for et in range(KT_E):
                        sl = tps[et // 4][:, (et % 4) * 128:(et % 4 + 1) * 128]
                        nc.tensor.transpose(
                            sl, hh_t[tt][:, et * 128:(et + 1) * 128],
                            ident[:, :])
                    for et in range(KT_E):
                        sl = tps[et // 4][:, (et % 4) * 128:(et % 4 + 1) * 128]
                        nc.vector.tensor_copy(
                            hT[:, et * S + tt * 128:et * S + (tt + 1) * 128], sl)

                # skewed: h-transposes of tile tt are emitted after the
                # SO GEMM of tile tt+1, so the PE never waits on LN1
                for tt in range(NT_B):
                    ch = []
                    for ec, n in ((0, 512), (512, 256)):
                        ps = (p_mm.tile([128, 512], F32, name="sops", tag="mm")
                              if n == 512 else
                              p_mm.tile([128, 256], F32, name="sops2", tag="mm"))
                        for k in range(KT_E):
                            nc.tensor.matmul(
                                ps[:, :n], xa[k][:, tt * 128:(tt + 1) * 128],
                                WSO[k][:, ec:ec + n],
                                start=(k == 0),
                                stop=(k == KT_E - 1 and not use_bso))
                        if use_bso:
                            nc.tensor.matmul(
                                ps[:, :n], ones[0:1, 0:128],
                                brow[4:5, ec:ec + n], start=False, stop=True)
                        ch.append((ps[:, :n], ec, n))
                    hh_t[tt] = h_pool.tile([128, E], BF16, name="hht", tag="h")
                    layernorm(ch, hh_t[tt], 0, use_g1, use_b1)
                    if tt > 0:
                        emit_htrans(tt - 1)
                emit_htrans(NT_B - 1)

                # ---- FFN + LN2 (full 512-token chunk: N=512 Wi GEMMs) ----
                ffT = [None] * FT
                for ft in range(FT):
                    ps = p_mm.tile([128, 512], F32, name="fips", tag="mm")
                    for k in range(KT_E):
                        nc.tensor.matmul(
                            ps[:, :], WI[k][:, ft * 128:(ft + 1) * 128],
                            hT[:, k * S:k * S + 512],
                            start=(k == 0), stop=(k == KT_E - 1))
                    ffT[ft] = fft_pool.tile([128, 512], BF16, name="fftt",
                                            tag="fft")
                    if use_bi:
                        nc.scalar.activation(ffT[ft][:, :], ps[:, :],
                                             AF.Gelu,
                                             bias=bic[:, ft:ft + 1])
                    else:
                        nc.scalar.activation(ffT[ft][:, :], ps[:, :],
                                             AF.Gelu)
                for tt in range(NT_B):
                    ch = []
                    for ec, n in ((0, 512), (512, 256)):
                        ps = (p_mm.tile([128, 512], F32, name="wops",
                                        tag="mm") if n == 512 else
                              p_mm.tile([128, 256], F32, name="wops2",
                                        tag="mm"))
                        for f in range(FT):
                            nc.tensor.matmul(
                                ps[:, :n],
                                ffT[f][:, tt * 128:(tt + 1) * 128],
                                WOUT[f][:, ec:ec + n],
                                start=(f == 0),
                                stop=(f == FT - 1 and not use_bout))
                        if use_bout:
                            nc.tensor.matmul(
                                ps[:, :n], ones[0:1, 0:128],
                                brow[5:6, ec:ec + n], start=False,
                                stop=True)
                        ch.append((ps[:, :n], ec, n))
                    otile = out_pool.tile([128, E], F32, name="ot",
                                          tag="outp")
                    resid = [hh_t[tt][:, ec:ec + n] for (_, ec, n) in ch]
                    layernorm(ch, otile, 1, use_g2, use_b2, resid=resid)
                    nc.gpsimd.dma_start(
                        d_out[t0 + tt * 128:t0 + (tt + 1) * 128, :],
                        otile[:, :])
    nc.compile()
    return nc


def _get_program(flags):
    key = ("prog", flags)
    if key not in _CACHE:
        _CACHE[key] = _build(flags)
    return _CACHE[key]


def kernel(x, mask, Wq, bq, Wk, bk, Wv, bv, Wo, bo,
           Wso, bso, gso, beso, Wi, bi, Wout, bout, gout, beout):
    from concourse.bass_utils import run_bass_kernel_spmd

    x = np.asarray(x, np.float32)
    mask = np.asarray(mask)
    sc = 1.0 / float(np.sqrt(np.float32(DK)))

    z = lambda a: not np.any(np.asarray(a))
    one = lambda a: bool(np.all(np.asarray(a) == 1.0))
    flags = (not z(bq), not z(bk), not z(bv), not z(bo), not z(bso),
             not z(bi), not z(bout),
             not one(gso), not z(beso), not one(gout), not z(beout))
    nc = _get_program(flags)

    wq_b = _bf(np.asarray(Wq, np.float32) * sc)
    wk_b, wv_b, wo_b, wso_b = _bf(Wk), _bf(Wv), _bf(Wo), _bf(Wso)
    wi_b, wout_b = _bf(Wi), _bf(Wout)
    identb = _bf(np.eye(128))
    onesr = _bf(np.ones((1, 512)))

    brow = np.zeros((7, FF), np.float32)
    brow[0, :E] = np.asarray(bq, np.float32) * sc
    for i, v in enumerate((bk, bv, bo, bso, bout)):
        brow[i + 1, :E] = v
    brow[6, :] = bi
    brow = _bf(brow)
    bicol = np.asarray(bi, np.float32).reshape(FF // 128, 128).T.copy()
    gbt = np.zeros((128, 4 * E), np.float32)
    for i, g in enumerate((gso, gout, beso, beout)):   # gamma1|gamma2|beta1|beta2
        gbt[:, i * E:(i + 1) * E] = np.broadcast_to(
            np.asarray(g, np.float32).reshape(1, E), (128, E))

    in_maps = []
    for c in range(NCORES):
        xs = x[c * BL:(c + 1) * BL].reshape(T, E)
        ms = np.asarray(mask[c * BL:(c + 1) * BL]).reshape(BL, S)
        mbias = _bf(np.where(ms == 0, np.float32(MASK_NEG),
                             np.float32(0.0)).reshape(1, T))
        in_maps.append({
            "x": _bf(xs), "wq": wq_b, "wk": wk_b, "wv": wv_b, "wo": wo_b,
            "wso": wso_b, "wi": wi_b, "wout": wout_b, "mbias": mbias,
            "ident": identb, "onesrow": onesr,
            "brow": brow, "bicol": bicol, "gb": gbt,
        })

    trace = os.environ.get("KERNEL_TRACE", "0") == "1"
    res = run_bass_kernel_spmd(nc, in_maps, core_ids=list(range(NCORES)),
                               trace=trace)
    if trace and res.exec_time_ns is not None:
        print(f"HW exec time: {res.exec_time_ns} ns")
        if res.instructions_and_trace is not None:
            print(f"trace: {res.instructions_and_trace[1]}")
    out = np.concatenate([r["out"].reshape(BL, S, E) for r in res.results],
                         axis=0)
    return np.ascontiguousarray(out.astype(np.float32))


# revision 26
# speedup vs baseline: 1.0719x; 1.0367x over previous
"""BERT-base encoder layer on 8 Trainium2 NeuronCores (Bass/Tile).

Sharding: data-parallel over batch. Full inputs [32, 512, 768] split into 8
shards of 4 batches (2048 tokens); every core runs the same NEFF on its shard
(SPMD, no collectives); host concatenates the outputs.

All GEMMs run on the PE in bf16 with fp32 PSUM accumulation; softmax and
layernorm statistics run in fp32. 1/sqrt(dk) is folded into Wq on the host.
The additive attention mask is applied by a K=1 rank-1 matmul accumulated
into the score PSUM (scores are bounded, so softmax needs no max-subtract).
The softmax normalization (1/rowsum) is folded into the PE transpose of the
probabilities by using diag(1/s) instead of the identity.
"""

import os
import numpy as np
import ml_dtypes

B, S, E, H, DK, FF = 32, 512, 768, 12, 64, 3072
NCORES = 8
BL = B // NCORES          # batches per core = 4
T = BL * S                # tokens per core = 2048
EPS = 1e-12
MASK_NEG = -87.0          # stays inside exp-table range; exp() == 0 in fp32

_CACHE = {}


def _bf(a):
    return np.ascontiguousarray(np.asarray(a, np.float32).astype(ml_dtypes.bfloat16))


def _build(flags):
    import concourse.bass as bass
    import concourse.bacc as bacc
    import concourse.mybir as mybir
    import concourse.tile as tile
    from contextlib import ExitStack

    (use_bq, use_bk, use_bv, use_bo, use_bso, use_bi, use_bout,
     use_g1, use_b1, use_g2, use_b2) = flags

    AF = mybir.ActivationFunctionType
    OP = mybir.AluOpType
    AX = mybir.AxisListType
    BF16 = mybir.dt.bfloat16
    F32 = mybir.dt.float32

    nc = bacc.Bacc("TRN2", target_bir_lowering=False)

    d_x = nc.dram_tensor("x", (T, E), BF16, kind="ExternalInput")
    d_wq = nc.dram_tensor("wq", (E, E), BF16, kind="ExternalInput")
    d_wk = nc.dram_tensor("wk", (E, E), BF16, kind="ExternalInput")
    d_wv = nc.dram_tensor("wv", (E, E), BF16, kind="ExternalInput")
    d_wo = nc.dram_tensor("wo", (E, E), BF16, kind="ExternalInput")
    d_wso = nc.dram_tensor("wso", (E, E), BF16, kind="ExternalInput")
    d_wi = nc.dram_tensor("wi", (E, FF), BF16, kind="ExternalInput")
    d_wout = nc.dram_tensor("wout", (FF, E), BF16, kind="ExternalInput")
    d_mb = nc.dram_tensor("mbias", (1, T), BF16, kind="ExternalInput")
    d_id = nc.dram_tensor("ident", (128, 128), BF16, kind="ExternalInput")
    d_ones = nc.dram_tensor("onesrow", (1, 512), BF16, kind="ExternalInput")
    # bias rows: 0=bq/8, 1=bk, 2=bv, 3=bo, 4=bso, 5=bout, 6=bi (full FF width)
    d_brow = nc.dram_tensor("brow", (7, FF), BF16, kind="ExternalInput")
    d_bic = nc.dram_tensor("bicol", (128, FF // 128), F32, kind="ExternalInput")
    # gamma1 | beta1 | gamma2 | beta2, each [128, 768] partition-broadcast
    d_gb = nc.dram_tensor("gb", (128, 4 * E), F32, kind="ExternalInput")
    d_out = nc.dram_tensor("out", (T, E), F32, kind="ExternalOutput")

    KT_E = E // 128    # 6
    NT_B = S // 128    # 4
    FT = FF // 128     # 24
    HP = H // 2        # 6

    need_gb = use_g1 or use_b1 or use_g2 or use_b2
    need_brow = use_bq or use_bk or use_bv or use_bo or use_bso or use_bout

    with ExitStack() as ctx:
        tc = ctx.enter_context(tile.TileContext(nc))

        p_mm = ctx.enter_context(tc.tile_pool(name="p_mm", bufs=8, space="PSUM"))

        c_pool = ctx.enter_context(tc.tile_pool(name="consts", bufs=1))
        xt_pool = ctx.enter_context(tc.tile_pool(name="xt", bufs=BL * KT_E))
        att_pool = ctx.enter_context(tc.tile_pool(name="attp", bufs=BL * KT_E + 2))

        ident = c_pool.tile_from(d_id[:, :], name="ident")
        ones = c_pool.tile_from(d_ones[:, :], name="ones")
        brow = c_pool.tile_from(d_brow[:, :], name="brow") if need_brow else None
        gb = c_pool.tile_from(d_gb[:, :], name="gb") if need_gb else None

        XT = {}    # (b, kt) -> [128, S] bf16, feature-major x
        ATT = {}   # (b, kt) -> [128, S] bf16, feature-major attention context

        # ================= superphase A: x^T, QKV, attention =================
        with ExitStack() as sa:
            a_pool = sa.enter_context(tc.tile_pool(name="a_consts", bufs=1))
            wq_pool = sa.enter_context(tc.tile_pool(name="wq", bufs=KT_E))
            wk_pool = sa.enter_context(tc.tile_pool(name="wk", bufs=KT_E))
            wv_pool = sa.enter_context(tc.tile_pool(name="wv", bufs=KT_E))
            xb_pool = sa.enter_context(tc.tile_pool(name="xb", bufs=4))
            qt_pool = sa.enter_context(tc.tile_pool(name="qt", bufs=KT_E + 6))
            kt_pool = sa.enter_context(tc.tile_pool(name="kt", bufs=KT_E + 6))
            v_pool = sa.enter_context(tc.tile_pool(name="v", bufs=NT_B + 2))
            p_pool = sa.enter_context(tc.tile_pool(name="pp", bufs=26))
            pt_pool = sa.enter_context(tc.tile_pool(name="pt", bufs=12))
            dg_pool = sa.enter_context(tc.tile_pool(name="dg", bufs=14))
            s_pool = sa.enter_context(tc.tile_pool(name="sa_s", bufs=16))

            mb = a_pool.tile_from(d_mb[:, :], name="mb")
            WQ = [wq_pool.tile_from(d_wq[k * 128:(k + 1) * 128, :], name="wqt")
                  for k in range(KT_E)]
            WK = [wk_pool.tile_from(d_wk[k * 128:(k + 1) * 128, :], name="wkt")
                  for k in range(KT_E)]
            WV = [wv_pool.tile_from(d_wv[k * 128:(k + 1) * 128, :], name="wvt")
                  for k in range(KT_E)]

            for b in range(BL):
                t0 = b * S
                # ---- x -> XT (feature-major), PE transpose ----
                for kt in range(KT_E):
                    XT[(b, kt)] = xt_pool.tile([128, S], BF16, name="xtt", tag="xt")
                for tt in range(NT_B):
                    xbt = xb_pool.tile([128, E], BF16, name="xbt", tag="xb")
                    nc.gpsimd.dma_start(
                        xbt[:, :], d_x[t0 + tt * 128:t0 + (tt + 1) * 128, :])
                    tps = [p_mm.tile([128, 512], BF16, name="xtp", tag="mm")
                           for _ in range(2)]
                    for et in range(KT_E):
                        sl = tps[et // 4][:, (et % 4) * 128:(et % 4 + 1) * 128]
                        nc.tensor.transpose(
                            sl, xbt[:, et * 128:(et + 1) * 128], ident[:, :])
                    for et in range(KT_E):
                        sl = tps[et // 4][:, (et % 4) * 128:(et % 4 + 1) * 128]
                        nc.vector.tensor_copy(
                            XT[(b, et)][:, tt * 128:(tt + 1) * 128], sl)

                # ---- Q/K projections (feature-major out) ----
                QT, KTt = [None] * KT_E, [None] * KT_E
                V = [None] * NT_B
                for Wt, dstl, pool, ub, brx, tg in (
                        (WQ, QT, qt_pool, use_bq, 0, "qt"),
                        (WK, KTt, kt_pool, use_bk, 1, "kt")):
                    for et in range(KT_E):
                        ps = p_mm.tile([128, S], F32, name="qkps", tag="mm")
                        for k in range(KT_E):
                            nc.tensor.matmul(
                                ps[:, :], Wt[k][:, et * 128:(et + 1) * 128],
                                XT[(b, k)][:, :],
                                start=(k == 0), stop=(k == KT_E - 1 and not ub))
                        if ub:
                            nc.tensor.matmul(
                                ps[:, :],
                                brow[brx:brx + 1, et * 128:(et + 1) * 128],
                                ones[0:1, 0:S], start=False, stop=True)
                        dstl[et] = pool.tile([128, S], BF16, name="qkt", tag=tg)
                        nc.vector.tensor_copy(dstl[et][:, :], ps[:, :])

                # ---- V projection (token-major out) ----
                for tt in range(NT_B):
                    V[tt] = v_pool.tile([128, E], BF16, name="vt", tag="v")
                    for ec, n in ((0, 512), (512, 256)):
                        ps = (p_mm.tile([128, 512], F32, name="vps", tag="mm")
                              if n == 512 else
                              p_mm.tile([128, 256], F32, name="vps2", tag="mm"))
                        for k in range(KT_E):
                            nc.tensor.matmul(
                                ps[:, :n], XT[(b, k)][:, tt * 128:(tt + 1) * 128],
                                WV[k][:, ec:ec + n],
                                start=(k == 0), stop=(k == KT_E - 1 and not use_bv))
                        if use_bv:
                            nc.tensor.matmul(
                                ps[:, :n], ones[0:1, 0:128],
                                brow[2:3, ec:ec + n], start=False, stop=True)
                        nc.vector.tensor_copy(V[tt][:, ec:ec + n], ps[:, :n])

                # ---- attention, per head-pair ----
                for kt in range(KT_E):
                    ATT[(b, kt)] = att_pool.tile([128, S], BF16, name="attt",
                                                 tag="attT")
                # PTn[(hp, hh)][p, jt*512 + q] = P[q, jt*128+p] / s_q
                PTn = {(hp, hh): pt_pool.tile([128, NT_B * 512], BF16,
                                              name="ptn", tag="pt")
                       for hp in range(HP) for hh in range(2)}
                # software-pipelined: group g = all 12 (hp, hh) chains of
                # q-tile g. Emit scores+exp+recip+diag of group g, but the
                # PT matmuls of group g only after group g+1's front half,
                # so the PE queue never blocks on the ACT/DVE round trip.
                front = {}   # it -> list of (hp, hh, pexp, dg)
                def emit_front(it):
                    chains = []
                    for hpg in range(0, HP, 2):   # sub-groups of 4 chains
                        quad = [(hp, hh) for hp in (hpg, hpg + 1)
                                for hh in range(2)]
                        pss = {}
                        for hp, hh in quad:       # QK MMs: rg01/rg23 pairs
                            o = hh * 64           # run concurrently on PE
                            ps = p_mm.tile([128, S], F32, name="scps",
                                           tag="mm")
                            nc.tensor.matmul(
                                ps[:, :],
                                QT[hp][o:o + 64, it * 128:(it + 1) * 128],
                                KTt[hp][o:o + 64, :], start=True, stop=False)
                            pss[(hp, hh)] = ps
                        for hp, hh in quad:       # mask rank-1 updates
                            nc.tensor.matmul(
                                pss[(hp, hh)][:, :], ones[0:1, 0:128],
                                mb[:, t0:t0 + S], start=False, stop=True)
                        for hp, hh in quad:       # softmax front
                            ps = pss[(hp, hh)]
                            pexp = p_pool.tile([128, S], BF16, name="pexp",
                                               tag="p")
                            ssum = s_pool.tile([128, 1], F32, name="ssum",
                                               tag="ss", bufs=64)
                            nc.scalar.activation(pexp[:, :], ps[:, :], AF.Exp,
                                                 accum_out=ssum[:, :])
                            rs = s_pool.tile([128, 1], F32, name="rsum",
                                             tag="rs")
                            nc.vector.reciprocal(rs[:, :], ssum[:, :])
                            dg = dg_pool.tile([128, 128], BF16, name="dgt",
                                              tag="dg")
                            nc.vector.tensor_scalar_mul(dg[:, :], ident[:, :],
                                                        rs[:, :])
                            chains.append((hp, hh, pexp, dg))
                    front[it] = chains

                def emit_back(it):
                    for hp, hh, pexp, dg in front.pop(it):
                        ptp = p_mm.tile([128, 512], F32, name="ptp", tag="mm")
                        for jc in range(NT_B):
                            nc.tensor.matmul(
                                ptp[:, jc * 128:(jc + 1) * 128],
                                pexp[:, jc * 128:(jc + 1) * 128], dg[:, :],
                                start=True, stop=True)
                        srcv = ptp.rearrange("p (j c) -> p j c", j=NT_B)
                        dst = PTn[(hp, hh)].rearrange("p (j c) -> p j c",
                                                      j=NT_B)
                        nc.vector.tensor_copy(
                            dst[:, :, it * 128:(it + 1) * 128], srcv)

                emit_front(0)
                for it in range(1, NT_B):
                    emit_front(it)
                    emit_back(it - 1)
                emit_back(NT_B - 1)
                for hp in range(HP):
                    ps = p_mm.tile([128, S], F32, name="avps", tag="mm")
                    for hh in range(2):
                        o = hh * 64
                        for jt in range(NT_B):
                            nc.tensor.matmul(
                                ps[o:o + 64, :],
                                V[jt][:, hp * 128 + o:hp * 128 + o + 64],
                                PTn[(hp, hh)][:, jt * 512:(jt + 1) * 512],
                                start=(jt == 0), stop=(jt == NT_B - 1),
                                tile_position=(0, o))
                    nc.vector.tensor_copy(ATT[(b, hp)][:, :], ps[:, :])

        # ============ superphase B: O-proj, SelfOutput LN, FFN, LN ===========
        with ExitStack() as sb:
            b_pool = sb.enter_context(tc.tile_pool(name="b_consts", bufs=1))
            wo_pool = sb.enter_context(tc.tile_pool(name="wo", bufs=KT_E))
            wso_pool = sb.enter_context(tc.tile_pool(name="wso", bufs=KT_E))
            wi_pool = sb.enter_context(tc.tile_pool(name="wi", bufs=KT_E))
            wout_pool = sb.enter_context(tc.tile_pool(name="wout", bufs=FT))
            h_pool = sb.enter_context(tc.tile_pool(name="h", bufs=NT_B + 1))
            ht_pool = sb.enter_context(tc.tile_pool(name="ht", bufs=2))
            fft_pool = sb.enter_context(tc.tile_pool(name="fft", bufs=FT + 2))
            sq_pool = sb.enter_context(tc.tile_pool(name="sq", bufs=2))
            rs_pool = sb.enter_context(tc.tile_pool(name="rsd", bufs=2))
            out_pool = sb.enter_context(tc.tile_pool(name="outp", bufs=2))
            t_pool = sb.enter_context(tc.tile_pool(name="sb_s", bufs=12))

            bic = b_pool.tile_from(d_bic[:, :], name="bic") if use_bi else None
            WO = [wo_pool.tile_from(d_wo[k * 128:(k + 1) * 128, :], name="wot")
                  for k in range(KT_E)]
            WSO = [wso_pool.tile_from(d_wso[k * 128:(k + 1) * 128, :], name="wsot")
                   for k in range(KT_E)]
            WI = [wi_pool.tile_from(d_wi[k * 128:(k + 1) * 128, :], name="wit")
                  for k in range(KT_E)]
            WOUT = [wout_pool.tile_from(d_wout[f * 128:(f + 1) * 128, :],
                                        name="woutt") for f in range(FT)]

            def layernorm(chunks, h_dst, gcol, use_g, use_bb, resid=None):
                """chunks: [(psum_ap, col0, n)]; h_dst: [128, E] fp32 out.
                resid: parallel list of sbuf fp32 APs added to psum first."""
                if resid is not None:
                    rtile = rs_pool.tile([128, E], F32, name="rt", tag="rsd")
                    for (ps, c0, n), rext in zip(chunks, resid):
                        nc.vector.scalar_tensor_tensor(
                            rtile[:, c0:c0 + n], ps, 1.0, rext,
                            op0=OP.mult, op1=OP.add)
                    srcs = [(rtile[:, c0:c0 + n], c0, n) for (_, c0, n) in chunks]
                else:
                    srcs = chunks
                s1 = t_pool.tile([128, 1], F32, name="s1", tag="s1")
                s1b = t_pool.tile([128, 1], F32, name="s1b", tag="s1b")
                nc.vector.reduce_sum(s1[:, :], srcs[0][0], axis=AX.X)
                nc.vector.reduce_sum(s1b[:, :], srcs[1][0], axis=AX.X)
                mu_n = t_pool.tile([128, 1], F32, name="mun", tag="mun")
                # mu_n = -(s1 + s1b)/E
                tmp = t_pool.tile([128, 1], F32, name="tmps", tag="tmps")
                nc.vector.scalar_tensor_tensor(
                    tmp[:, :], s1[:, :], 1.0, s1b[:, :], op0=OP.mult, op1=OP.add)
                nc.vector.tensor_scalar_mul(mu_n[:, :], tmp[:, :], -1.0 / E)
                ss = t_pool.tile([128, 1], F32, name="ssa", tag="ssa", bufs=34)
                ssb = t_pool.tile([128, 1], F32, name="ssb", tag="ssb", bufs=34)
                for (src, c0, n), acc in zip(srcs, (ss, ssb)):
                    sq = sq_pool.tile([128, 512], BF16, name="sqt", tag="sq")
                    nc.scalar.activation(sq[:, :n], src, AF.Square,
                                         accum_out=acc[:, :])
                # var = (ss+ssb)/E - mu^2 ; rstd = 1/sqrt(var + eps)
                musq = t_pool.tile([128, 1], F32, name="musq", tag="musq")
                nc.vector.scalar_tensor_tensor(
                    musq[:, :], mu_n[:, :], 1.0, mu_n[:, :],
                    op0=OP.mult, op1=OP.mult)
                veps = t_pool.tile([128, 1], F32, name="veps", tag="veps")
                nc.vector.scalar_tensor_tensor(
                    veps[:, :], ss[:, :], 1.0, ssb[:, :],
                    op0=OP.mult, op1=OP.add)
                veps2 = t_pool.tile([128, 1], F32, name="veps2", tag="veps2")
                nc.vector.tensor_scalar(
                    veps2[:, :], veps[:, :], 1.0 / E, EPS,
                    op0=OP.mult, op1=OP.add)
                veps3 = t_pool.tile([128, 1], F32, name="veps3", tag="veps3")
                nc.vector.scalar_tensor_tensor(
                    veps3[:, :], musq[:, :], -1.0, veps2[:, :],
                    op0=OP.mult, op1=OP.add)
                sd = t_pool.tile([128, 1], F32, name="sd", tag="sd")
                nc.scalar.sqrt(sd[:, :], veps3[:, :])
                rstd = t_pool.tile([128, 1], F32, name="rstd", tag="rstd")
                nc.vector.reciprocal(rstd[:, :], sd[:, :])
                for (src, c0, n) in srcs:
                    nc.vector.tensor_scalar(
                        h_dst[:, c0:c0 + n], src, mu_n[:, :], rstd[:, :],
                        op0=OP.add, op1=OP.mult)
                if use_g:
                    nc.vector.scalar_tensor_tensor(
                        h_dst[:, :], h_dst[:, :], 1.0,
                        gb[:, gcol * E:(gcol + 1) * E], op0=OP.mult, op1=OP.mult)
                if use_bb:
                    nc.vector.scalar_tensor_tensor(
                        h_dst[:, :], h_dst[:, :], 1.0,
                        gb[:, (gcol + 2) * E:(gcol + 3) * E],
                        op0=OP.mult, op1=OP.add)

            for b in range(BL):
                t0 = b * S
                # ---- O-projection + residual -> xa (feature-major bf16) ----
                xa = [None] * KT_E
                for et in range(KT_E):
                    ps = p_mm.tile([128, S], F32, name="ops", tag="mm")
                    for k in range(KT_E):
                        nc.tensor.matmul(
                            ps[:, :], WO[k][:, et * 128:(et + 1) * 128],
                            ATT[(b, k)][:, :],
                            start=(k == 0), stop=(k == KT_E - 1 and not use_bo))
                    if use_bo:
                        nc.tensor.matmul(
                            ps[:, :], brow[3:4, et * 128:(et + 1) * 128],
                            ones[0:1, 0:S], start=False, stop=True)
                    xa[et] = att_pool.tile([128, S], BF16, name="xat", tag="attT")
                    nc.vector.scalar_tensor_tensor(
                        xa[et][:, :], ps[:, :], 1.0, XT[(b, et)][:, :],
                        op0=OP.mult, op1=OP.add)

                # ---- SelfOutput GEMM + LN1 -> h (token-major fp32), hT ----
                hh_t = [None] * NT_B
                hT = ht_pool.tile([128, KT_E * S], BF16, name="htt", tag="ht")

                def emit_htrans(tt):
                    tps = [p_mm.tile([128, 512], BF16, name="htp", tag="mm")
                           for _ in range(2)]
                    for et in range(KT_E):
                        sl = tps[et // 4][:, (et % 4) * 128:(et % 4 + 1) * 128]
                        nc.tensor.transpose(
                            sl, hh_t[tt][:, et * 128:(et + 1) * 128],
                            ident[:, :])
                    for et in range(KT_E):
                        sl = tps[et // 4][:, (et % 4) * 128:(et % 4 + 1) * 128]
                        nc.vector.tensor_copy(
                            hT[:, et * S + tt * 128:et * S + (tt + 1) * 128], sl)

                # skewed: h-transposes of tile tt are emitted after the
                # SO GEMM of tile tt+1, so the PE never waits on LN1
                for tt in range(NT_B):
                    ch = []
                    for ec, n in ((0, 512), (512, 256)):
                        ps = (p_mm.tile([128, 512], F32, name="sops", tag="mm")
                              if n == 512 else
                              p_mm.tile([128, 256], F32, name="sops2", tag="mm"))
                        for k in range(KT_E):
                            nc.tensor.matmul(
                                ps[:, :n], xa[k][:, tt * 128:(tt + 1) * 128],
                                WSO[k][:, ec:ec + n],
                                start=(k == 0),
                                stop=(k == KT_E - 1 and not use_bso))
                        if use_bso:
                            nc.tensor.matmul(
                                ps[:, :n], ones[0:1, 0:128],
                                brow[4:5, ec:ec + n], start=False, stop=True)
                        ch.append((ps[:, :n], ec, n))
                    hh_t[tt] = h_pool.tile([128, E], BF16, name="hht", tag="h")
                    layernorm(ch, hh_t[tt], 0, use_g1, use_b1)
                    if tt > 0:
                        emit_htrans(tt - 1)
                emit_htrans(NT_B - 1)

                # ---- FFN + LN2 (full 512-token chunk: N=512 Wi GEMMs) ----
                ffT = [None] * FT
                for ft in range(FT):
                    ps = p_mm.tile([128, 512], F32, name="fips", tag="mm")
                    for k in range(KT_E):
                        nc.tensor.matmul(
                            ps[:, :], WI[k][:, ft * 128:(ft + 1) * 128],
                            hT[:, k * S:k * S + 512],
                            start=(k == 0), stop=(k == KT_E - 1))
                    ffT[ft] = fft_pool.tile([128, 512], BF16, name="fftt",
                                            tag="fft")
                    if use_bi:
                        nc.scalar.activation(ffT[ft][:, :], ps[:, :],
                                             AF.Gelu,
                                             bias=bic[:, ft:ft + 1])
                    else:
                        nc.scalar.activation(ffT[ft][:, :], ps[:, :],
                                             AF.Gelu)
                for tt in range(NT_B):
                    ch = []
                    for ec, n in ((0, 512), (512, 256)):
                        ps = (p_mm.tile([128, 512], F32, name="wops",
                                        tag="mm") if n == 512 else
                              p_mm.tile([128, 256], F32, name="wops2",
                                        tag="mm"))
                        for f in range(FT):
                            nc.tensor.matmul(
                                ps[:, :n],
                                ffT[f][:, tt * 128:(tt + 1) * 128],
                                WOUT[f][:, ec:ec + n],
                                start=(f == 0),
                                stop=(f == FT - 1 and not use_bout))
                        if use_bout:
                            nc.tensor.matmul(
                                ps[:, :n], ones[0:1, 0:128],
                                brow[5:6, ec:ec + n], start=False,
                                stop=True)
                        ch.append((ps[:, :n], ec, n))
                    otile = out_pool.tile([128, E], F32, name="ot",
                                          tag="outp")
                    resid = [hh_t[tt][:, ec:ec + n] for (_, ec, n) in ch]
                    layernorm(ch, otile, 1, use_g2, use_b2, resid=resid)
                    nc.gpsimd.dma_start(
                        d_out[t0 + tt * 128:t0 + (tt + 1) * 128, :],
                        otile[:, :])
    nc.compile()
    return nc


def _get_program(flags):
    key = ("prog", flags)
    if key not in _CACHE:
        _CACHE[key] = _build(flags)
    return _CACHE[key]


def kernel(x, mask, Wq, bq, Wk, bk, Wv, bv, Wo, bo,
           Wso, bso, gso, beso, Wi, bi, Wout, bout, gout, beout):
    from concourse.bass_utils import run_bass_kernel_spmd

    x = np.asarray(x, np.float32)
    mask = np.asarray(mask)
    sc = 1.0 / float(np.sqrt(np.float32(DK)))

    z = lambda a: not np.any(np.asarray(a))
    one = lambda a: bool(np.all(np.asarray(a) == 1.0))
    flags = (not z(bq), not z(bk), not z(bv), not z(bo), not z(bso),
             not z(bi), not z(bout),
             not one(gso), not z(beso), not one(gout), not z(beout))
    nc = _get_program(flags)

    wq_b = _bf(np.asarray(Wq, np.float32) * sc)
    wk_b, wv_b, wo_b, wso_b = _bf(Wk), _bf(Wv), _bf(Wo), _bf(Wso)
    wi_b, wout_b = _bf(Wi), _bf(Wout)
    identb = _bf(np.eye(128))
    onesr = _bf(np.ones((1, 512)))

    brow = np.zeros((7, FF), np.float32)
    brow[0, :E] = np.asarray(bq, np.float32) * sc
    for i, v in enumerate((bk, bv, bo, bso, bout)):
        brow[i + 1, :E] = v
    brow[6, :] = bi
    brow = _bf(brow)
    bicol = np.asarray(bi, np.float32).reshape(FF // 128, 128).T.copy()
    gbt = np.zeros((128, 4 * E), np.float32)
    for i, g in enumerate((gso, gout, beso, beout)):   # gamma1|gamma2|beta1|beta2
        gbt[:, i * E:(i + 1) * E] = np.broadcast_to(
            np.asarray(g, np.float32).reshape(1, E), (128, E))

    in_maps = []
    for c in range(NCORES):
        xs = x[c * BL:(c + 1) * BL].reshape(T, E)
        ms = np.asarray(mask[c * BL:(c + 1) * BL]).reshape(BL, S)
        mbias = _bf(np.where(ms == 0, np.float32(MASK_NEG),
                             np.float32(0.0)).reshape(1, T))
        in_maps.append({
            "x": _bf(xs), "wq": wq_b, "wk": wk_b, "wv": wv_b, "wo": wo_b,
            "wso": wso_b, "wi": wi_b, "wout": wout_b, "mbias": mbias,
            "ident": identb, "onesrow": onesr,
            "brow": brow, "bicol": bicol, "gb": gbt,
        })

    trace = os.environ.get("KERNEL_TRACE", "0") == "1"
    res = run_bass_kernel_spmd(nc, in_maps, core_ids=list(range(NCORES)),
                               trace=trace)
    if trace and res.exec_time_ns is not None:
        print(f"HW exec time: {res.exec_time_ns} ns")
        if res.instructions_and_trace is not None:
            print(f"trace: {res.instructions_and_trace[1]}")
    out = np.concatenate([r["out"].reshape(BL, S, E) for r in res.results],
                         axis=0)
    return np.ascontiguousarray(out.astype(np.float32))


# revision 27
# speedup vs baseline: 1.0745x; 1.0024x over previous
"""BERT-base encoder layer on 8 Trainium2 NeuronCores (Bass/Tile).

Sharding: data-parallel over batch. Full inputs [32, 512, 768] split into 8
shards of 4 batches (2048 tokens); every core runs the same NEFF on its shard
(SPMD, no collectives); host concatenates the outputs.

All GEMMs run on the PE in bf16 with fp32 PSUM accumulation; softmax and
layernorm statistics run in fp32. 1/sqrt(dk) is folded into Wq on the host.
The additive attention mask is applied by a K=1 rank-1 matmul accumulated
into the score PSUM (scores are bounded, so softmax needs no max-subtract).
The softmax normalization (1/rowsum) is folded into the PE transpose of the
probabilities by using diag(1/s) instead of the identity.
"""

import os
import numpy as np
import ml_dtypes

B, S, E, H, DK, FF = 32, 512, 768, 12, 64, 3072
NCORES = 8
BL = B // NCORES          # batches per core = 4
T = BL * S                # tokens per core = 2048
EPS = 1e-12
MASK_NEG = -87.0          # stays inside exp-table range; exp() == 0 in fp32

_CACHE = {}


def _bf(a):
    return np.ascontiguousarray(np.asarray(a, np.float32).astype(ml_dtypes.bfloat16))


def _build(flags):
    import concourse.bass as bass
    import concourse.bacc as bacc
    import concourse.mybir as mybir
    import concourse.tile as tile
    from contextlib import ExitStack

    (use_bq, use_bk, use_bv, use_bo, use_bso, use_bi, use_bout,
     use_g1, use_b1, use_g2, use_b2) = flags

    AF = mybir.ActivationFunctionType
    OP = mybir.AluOpType
    AX = mybir.AxisListType
    BF16 = mybir.dt.bfloat16
    F32 = mybir.dt.float32

    nc = bacc.Bacc("TRN2", target_bir_lowering=False)

    d_x = nc.dram_tensor("x", (T, E), BF16, kind="ExternalInput")
    d_wq = nc.dram_tensor("wq", (E, E), BF16, kind="ExternalInput")
    d_wk = nc.dram_tensor("wk", (E, E), BF16, kind="ExternalInput")
    d_wv = nc.dram_tensor("wv", (E, E), BF16, kind="ExternalInput")
    d_wo = nc.dram_tensor("wo", (E, E), BF16, kind="ExternalInput")
    d_wso = nc.dram_tensor("wso", (E, E), BF16, kind="ExternalInput")
    d_wi = nc.dram_tensor("wi", (E, FF), BF16, kind="ExternalInput")
    d_wout = nc.dram_tensor("wout", (FF, E), BF16, kind="ExternalInput")
    d_mb = nc.dram_tensor("mbias", (1, T), BF16, kind="ExternalInput")
    d_id = nc.dram_tensor("ident", (128, 128), BF16, kind="ExternalInput")
    d_ones = nc.dram_tensor("onesrow", (1, 512), BF16, kind="ExternalInput")
    # bias rows: 0=bq/8, 1=bk, 2=bv, 3=bo, 4=bso, 5=bout, 6=bi (full FF width)
    d_brow = nc.dram_tensor("brow", (7, FF), BF16, kind="ExternalInput")
    d_bic = nc.dram_tensor("bicol", (128, FF // 128), F32, kind="ExternalInput")
    # gamma1 | beta1 | gamma2 | beta2, each [128, 768] partition-broadcast
    d_gb = nc.dram_tensor("gb", (128, 4 * E), F32, kind="ExternalInput")
    d_out = nc.dram_tensor("out", (T, E), F32, kind="ExternalOutput")

    KT_E = E // 128    # 6
    NT_B = S // 128    # 4
    FT = FF // 128     # 24
    HP = H // 2        # 6

    need_gb = use_g1 or use_b1 or use_g2 or use_b2
    need_brow = use_bq or use_bk or use_bv or use_bo or use_bso or use_bout

    with ExitStack() as ctx:
        tc = ctx.enter_context(tile.TileContext(nc))

        p_mm = ctx.enter_context(tc.tile_pool(name="p_mm", bufs=8, space="PSUM"))

        c_pool = ctx.enter_context(tc.tile_pool(name="consts", bufs=1))
        xt_pool = ctx.enter_context(tc.tile_pool(name="xt", bufs=BL * KT_E))
        att_pool = ctx.enter_context(tc.tile_pool(name="attp", bufs=BL * KT_E + 2))

        ident = c_pool.tile_from(d_id[:, :], name="ident")
        ones = c_pool.tile_from(d_ones[:, :], name="ones")
        brow = c_pool.tile_from(d_brow[:, :], name="brow") if need_brow else None
        gb = c_pool.tile_from(d_gb[:, :], name="gb") if need_gb else None

        XT = {}    # (b, kt) -> [128, S] bf16, feature-major x
        ATT = {}   # (b, kt) -> [128, S] bf16, feature-major attention context

        # ================= superphase A: x^T, QKV, attention =================
        with ExitStack() as sa:
            a_pool = sa.enter_context(tc.tile_pool(name="a_consts", bufs=1))
            wq_pool = sa.enter_context(tc.tile_pool(name="wq", bufs=KT_E))
            wk_pool = sa.enter_context(tc.tile_pool(name="wk", bufs=KT_E))
            wv_pool = sa.enter_context(tc.tile_pool(name="wv", bufs=KT_E))
            xb_pool = sa.enter_context(tc.tile_pool(name="xb", bufs=4))
            qt_pool = sa.enter_context(tc.tile_pool(name="qt", bufs=KT_E + 6))
            kt_pool = sa.enter_context(tc.tile_pool(name="kt", bufs=KT_E + 6))
            v_pool = sa.enter_context(tc.tile_pool(name="v", bufs=NT_B + 2))
            p_pool = sa.enter_context(tc.tile_pool(name="pp", bufs=26))
            pt_pool = sa.enter_context(tc.tile_pool(name="pt", bufs=12))
            dg_pool = sa.enter_context(tc.tile_pool(name="dg", bufs=14))
            s_pool = sa.enter_context(tc.tile_pool(name="sa_s", bufs=16))

            mb = a_pool.tile_from(d_mb[:, :], name="mb")
            WQ = [wq_pool.tile_from(d_wq[k * 128:(k + 1) * 128, :], name="wqt")
                  for k in range(KT_E)]
            WK = [wk_pool.tile_from(d_wk[k * 128:(k + 1) * 128, :], name="wkt")
                  for k in range(KT_E)]
            WV = [wv_pool.tile_from(d_wv[k * 128:(k + 1) * 128, :], name="wvt")
                  for k in range(KT_E)]

            for b in range(BL):
                t0 = b * S
                # ---- x -> XT (feature-major), PE transpose ----
                for kt in range(KT_E):
                    XT[(b, kt)] = xt_pool.tile([128, S], BF16, name="xtt", tag="xt")
                for tt in range(NT_B):
                    xbt = xb_pool.tile([128, E], BF16, name="xbt", tag="xb")
                    nc.gpsimd.dma_start(
                        xbt[:, :], d_x[t0 + tt * 128:t0 + (tt + 1) * 128, :])
                    tps = [p_mm.tile([128, 512], BF16, name="xtp", tag="mm")
                           for _ in range(2)]
                    for et in range(KT_E):
                        sl = tps[et // 4][:, (et % 4) * 128:(et % 4 + 1) * 128]
                        nc.tensor.transpose(
                            sl, xbt[:, et * 128:(et + 1) * 128], ident[:, :])
                    for et in range(KT_E):
                        sl = tps[et // 4][:, (et % 4) * 128:(et % 4 + 1) * 128]
                        nc.vector.tensor_copy(
                            XT[(b, et)][:, tt * 128:(tt + 1) * 128], sl)

                # ---- Q/K projections (feature-major out) ----
                QT, KTt = [None] * KT_E, [None] * KT_E
                V = [None] * NT_B
                for Wt, dstl, pool, ub, brx, tg in (
                        (WQ, QT, qt_pool, use_bq, 0, "qt"),
                        (WK, KTt, kt_pool, use_bk, 1, "kt")):
                    for et in range(KT_E):
                        ps = p_mm.tile([128, S], F32, name="qkps", tag="mm")
                        for k in range(KT_E):
                            nc.tensor.matmul(
                                ps[:, :], Wt[k][:, et * 128:(et + 1) * 128],
                                XT[(b, k)][:, :],
                                start=(k == 0), stop=(k == KT_E - 1 and not ub))
                        if ub:
                            nc.tensor.matmul(
                                ps[:, :],
                                brow[brx:brx + 1, et * 128:(et + 1) * 128],
                                ones[0:1, 0:S], start=False, stop=True)
                        dstl[et] = pool.tile([128, S], BF16, name="qkt", tag=tg)
                        nc.vector.tensor_copy(dstl[et][:, :], ps[:, :])

                # ---- V projection (token-major out) ----
                for tt in range(NT_B):
                    V[tt] = v_pool.tile([128, E], BF16, name="vt", tag="v")
                    for ec, n in ((0, 512), (512, 256)):
                        ps = (p_mm.tile([128, 512], F32, name="vps", tag="mm")
                              if n == 512 else
                              p_mm.tile([128, 256], F32, name="vps2", tag="mm"))
                        for k in range(KT_E):
                            nc.tensor.matmul(
                                ps[:, :n], XT[(b, k)][:, tt * 128:(tt + 1) * 128],
                                WV[k][:, ec:ec + n],
                                start=(k == 0), stop=(k == KT_E - 1 and not use_bv))
                        if use_bv:
                            nc.tensor.matmul(
                                ps[:, :n], ones[0:1, 0:128],
                                brow[2:3, ec:ec + n], start=False, stop=True)
                        nc.vector.tensor_copy(V[tt][:, ec:ec + n], ps[:, :n])

                # ---- attention, per head-pair ----
                for kt in range(KT_E):
                    ATT[(b, kt)] = att_pool.tile([128, S], BF16, name="attt",
                                                 tag="attT")
                # PTn[(hp, hh)][p, jt*512 + q] = P[q, jt*128+p] / s_q
                PTn = {(hp, hh): pt_pool.tile([128, NT_B * 512], BF16,
                                              name="ptn", tag="pt")
                       for hp in range(HP) for hh in range(2)}
                # software-pipelined: group g = all 12 (hp, hh) chains of
                # q-tile g. Emit scores+exp+recip+diag of group g, but the
                # PT matmuls of group g only after group g+1's front half,
                # so the PE queue never blocks on the ACT/DVE round trip.
                front = {}   # it -> list of (hp, hh, pexp, dg)
                def emit_front(it):
                    chains = []
                    for hpg in range(0, HP, 2):   # sub-groups of 4 chains
                        quad = [(hp, hh) for hp in (hpg, hpg + 1)
                                for hh in range(2)]
                        pss = {}
                        for hp, hh in quad:       # QK MMs: rg01/rg23 pairs
                            o = hh * 64           # run concurrently on PE
                            ps = p_mm.tile([128, S], F32, name="scps",
                                           tag="mm")
                            nc.tensor.matmul(
                                ps[:, :],
                                QT[hp][o:o + 64, it * 128:(it + 1) * 128],
                                KTt[hp][o:o + 64, :], start=True, stop=False)
                            pss[(hp, hh)] = ps
                        for hp, hh in quad:       # mask rank-1 updates
                            nc.tensor.matmul(
                                pss[(hp, hh)][:, :], ones[0:1, 0:128],
                                mb[:, t0:t0 + S], start=False, stop=True)
                        for hp, hh in quad:       # softmax front
                            ps = pss[(hp, hh)]
                            pexp = p_pool.tile([128, S], BF16, name="pexp",
                                               tag="p")
                            ssum = s_pool.tile([128, 1], F32, name="ssum",
                                               tag="ss", bufs=64)
                            nc.scalar.activation(pexp[:, :], ps[:, :], AF.Exp,
                                                 accum_out=ssum[:, :])
                            rs = s_pool.tile([128, 1], F32, name="rsum",
                                             tag="rs")
                            nc.vector.reciprocal(rs[:, :], ssum[:, :])
                            dg = dg_pool.tile([128, 128], BF16, name="dgt",
                                              tag="dg")
                            nc.vector.tensor_scalar_mul(dg[:, :], ident[:, :],
                                                        rs[:, :])
                            chains.append((hp, hh, pexp, dg))
                    front[it] = chains

                def emit_back(it):
                    for hp, hh, pexp, dg in front.pop(it):
                        ptp = p_mm.tile([128, 512], F32, name="ptp", tag="mm")
                        for jc in range(NT_B):
                            nc.tensor.matmul(
                                ptp[:, jc * 128:(jc + 1) * 128],
                                pexp[:, jc * 128:(jc + 1) * 128], dg[:, :],
                                start=True, stop=True)
                        srcv = ptp.rearrange("p (j c) -> p j c", j=NT_B)
                        dst = PTn[(hp, hh)].rearrange("p (j c) -> p j c",
                                                      j=NT_B)
                        nc.vector.tensor_copy(
                            dst[:, :, it * 128:(it + 1) * 128], srcv)

                emit_front(0)
                for it in range(1, NT_B):
                    emit_front(it)
                    emit_back(it - 1)
                emit_back(NT_B - 1)
                for hp in range(HP):
                    ps = p_mm.tile([128, S], F32, name="avps", tag="mm")
                    for jt in range(NT_B):      # interleave col-groups so the
                        for hh in range(2):     # hh=0/hh=1 MMs overlap on PE
                            o = hh * 64
                            nc.tensor.matmul(
                                ps[o:o + 64, :],
                                V[jt][:, hp * 128 + o:hp * 128 + o + 64],
                                PTn[(hp, hh)][:, jt * 512:(jt + 1) * 512],
                                start=(jt == 0), stop=(jt == NT_B - 1),
                                tile_position=(0, o))
                    nc.vector.tensor_copy(ATT[(b, hp)][:, :], ps[:, :])

        # ============ superphase B: O-proj, SelfOutput LN, FFN, LN ===========
        with ExitStack() as sb:
            b_pool = sb.enter_context(tc.tile_pool(name="b_consts", bufs=1))
            wo_pool = sb.enter_context(tc.tile_pool(name="wo", bufs=KT_E))
            wso_pool = sb.enter_context(tc.tile_pool(name="wso", bufs=KT_E))
            wi_pool = sb.enter_context(tc.tile_pool(name="wi", bufs=KT_E))
            wout_pool = sb.enter_context(tc.tile_pool(name="wout", bufs=FT))
            h_pool = sb.enter_context(tc.tile_pool(name="h", bufs=NT_B + 1))
            ht_pool = sb.enter_context(tc.tile_pool(name="ht", bufs=2))
            fft_pool = sb.enter_context(tc.tile_pool(name="fft", bufs=FT + 2))
            sq_pool = sb.enter_context(tc.tile_pool(name="sq", bufs=2))
            rs_pool = sb.enter_context(tc.tile_pool(name="rsd", bufs=2))
            out_pool = sb.enter_context(tc.tile_pool(name="outp", bufs=2))
            t_pool = sb.enter_context(tc.tile_pool(name="sb_s", bufs=12))

            bic = b_pool.tile_from(d_bic[:, :], name="bic") if use_bi else None
            WO = [wo_pool.tile_from(d_wo[k * 128:(k + 1) * 128, :], name="wot")
                  for k in range(KT_E)]
            WSO = [wso_pool.tile_from(d_wso[k * 128:(k + 1) * 128, :], name="wsot")
                   for k in range(KT_E)]
            WI = [wi_pool.tile_from(d_wi[k * 128:(k + 1) * 128, :], name="wit")
                  for k in range(KT_E)]
            WOUT = [wout_pool.tile_from(d_wout[f * 128:(f + 1) * 128, :],
                                        name="woutt") for f in range(FT)]

            def layernorm(chunks, h_dst, gcol, use_g, use_bb, resid=None):
                """chunks: [(psum_ap, col0, n)]; h_dst: [128, E] fp32 out.
                resid: parallel list of sbuf fp32 APs added to psum first."""
                if resid is not None:
                    rtile = rs_pool.tile([128, E], F32, name="rt", tag="rsd")
                    for (ps, c0, n), rext in zip(chunks, resid):
                        nc.vector.scalar_tensor_tensor(
                            rtile[:, c0:c0 + n], ps, 1.0, rext,
                            op0=OP.mult, op1=OP.add)
                    srcs = [(rtile[:, c0:c0 + n], c0, n) for (_, c0, n) in chunks]
                else:
                    srcs = chunks
                s1 = t_pool.tile([128, 1], F32, name="s1", tag="s1")
                s1b = t_pool.tile([128, 1], F32, name="s1b", tag="s1b")
                nc.vector.reduce_sum(s1[:, :], srcs[0][0], axis=AX.X)
                nc.vector.reduce_sum(s1b[:, :], srcs[1][0], axis=AX.X)
                mu_n = t_pool.tile([128, 1], F32, name="mun", tag="mun")
                # mu_n = -(s1 + s1b)/E
                tmp = t_pool.tile([128, 1], F32, name="tmps", tag="tmps")
                nc.vector.scalar_tensor_tensor(
                    tmp[:, :], s1[:, :], 1.0, s1b[:, :], op0=OP.mult, op1=OP.add)
                nc.vector.tensor_scalar_mul(mu_n[:, :], tmp[:, :], -1.0 / E)
                ss = t_pool.tile([128, 1], F32, name="ssa", tag="ssa", bufs=34)
                ssb = t_pool.tile([128, 1], F32, name="ssb", tag="ssb", bufs=34)
                for (src, c0, n), acc in zip(srcs, (ss, ssb)):
                    sq = sq_pool.tile([128, 512], BF16, name="sqt", tag="sq")
                    nc.scalar.activation(sq[:, :n], src, AF.Square,
                                         accum_out=acc[:, :])
                # var = (ss+ssb)/E - mu^2 ; rstd = 1/sqrt(var + eps)
                musq = t_pool.tile([128, 1], F32, name="musq", tag="musq")
                nc.vector.scalar_tensor_tensor(
                    musq[:, :], mu_n[:, :], 1.0, mu_n[:, :],
                    op0=OP.mult, op1=OP.mult)
                veps = t_pool.tile([128, 1], F32, name="veps", tag="veps")
                nc.vector.scalar_tensor_tensor(
                    veps[:, :], ss[:, :], 1.0, ssb[:, :],
                    op0=OP.mult, op1=OP.add)
                veps2 = t_pool.tile([128, 1], F32, name="veps2", tag="veps2")
                nc.vector.tensor_scalar(
                    veps2[:, :], veps[:, :], 1.0 / E, EPS,
                    op0=OP.mult, op1=OP.add)
                veps3 = t_pool.tile([128, 1], F32, name="veps3", tag="veps3")
                nc.vector.scalar_tensor_tensor(
                    veps3[:, :], musq[:, :], -1.0, veps2[:, :],
                    op0=OP.mult, op1=OP.add)
                sd = t_pool.tile([128, 1], F32, name="sd", tag="sd")
                nc.scalar.sqrt(sd[:, :], veps3[:, :])
                rstd = t_pool.tile([128, 1], F32, name="rstd", tag="rstd")
                nc.vector.reciprocal(rstd[:, :], sd[:, :])
                for (src, c0, n) in srcs:
                    nc.vector.tensor_scalar(
                        h_dst[:, c0:c0 + n], src, mu_n[:, :], rstd[:, :],
                        op0=OP.add, op1=OP.mult)
                if use_g:
                    nc.vector.scalar_tensor_tensor(
                        h_dst[:, :], h_dst[:, :], 1.0,
                        gb[:, gcol * E:(gcol + 1) * E], op0=OP.mult, op1=OP.mult)
                if use_bb:
                    nc.vector.scalar_tensor_tensor(
                        h_dst[:, :], h_dst[:, :], 1.0,
                        gb[:, (gcol + 2) * E:(gcol + 3) * E],
                        op0=OP.mult, op1=OP.add)

            for b in range(BL):
                t0 = b * S
                # ---- O-projection + residual -> xa (feature-major bf16) ----
                xa = [None] * KT_E
                for et in range(KT_E):
                    ps = p_mm.tile([128, S], F32, name="ops", tag="mm")
                    for k in range(KT_E):
                        nc.tensor.matmul(
                            ps[:, :], WO[k][:, et * 128:(et + 1) * 128],
                            ATT[(b, k)][:, :],
                            start=(k == 0), stop=(k == KT_E - 1 and not use_bo))
                    if use_bo:
                        nc.tensor.matmul(
                            ps[:, :], brow[3:4, et * 128:(et + 1) * 128],
                            ones[0:1, 0:S], start=False, stop=True)
                    xa[et] = att_pool.tile([128, S], BF16, name="xat", tag="attT")
                    nc.vector.scalar_tensor_tensor(
                        xa[et][:, :], ps[:, :], 1.0, XT[(b, et)][:, :],
                        op0=OP.mult, op1=OP.add)

                # ---- SelfOutput GEMM + LN1 -> h (token-major fp32), hT ----
                hh_t = [None] * NT_B
                hT = ht_pool.tile([128, KT_E * S], BF16, name="htt", tag="ht")

                def emit_htrans(tt):
                    tps = [p_mm.tile([128, 512], BF16, name="htp", tag="mm")
                           for _ in range(2)]
                    for et in range(KT_E):
                        sl = tps[et // 4][:, (et % 4) * 128:(et % 4 + 1) * 128]
                        nc.tensor.transpose(
                            sl, hh_t[tt][:, et * 128:(et + 1) * 128],
                            ident[:, :])
                    for et in range(KT_E):
                        sl = tps[et // 4][:, (et % 4) * 128:(et % 4 + 1) * 128]
                        nc.vector.tensor_copy(
                            hT[:, et * S + tt * 128:et * S + (tt + 1) * 128], sl)

                # skewed: h-transposes of tile tt are emitted after the
                # SO GEMM of tile tt+1, so the PE never waits on LN1
                for tt in range(NT_B):
                    ch = []
                    for ec, n in ((0, 512), (512, 256)):
                        ps = (p_mm.tile([128, 512], F32, name="sops", tag="mm")
                              if n == 512 else
                              p_mm.tile([128, 256], F32, name="sops2", tag="mm"))
                        for k in range(KT_E):
                            nc.tensor.matmul(
                                ps[:, :n], xa[k][:, tt * 128:(tt + 1) * 128],
                                WSO[k][:, ec:ec + n],
                                start=(k == 0),
                                stop=(k == KT_E - 1 and not use_bso))
                        if use_bso:
                            nc.tensor.matmul(
                                ps[:, :n], ones[0:1, 0:128],
                                brow[4:5, ec:ec + n], start=False, stop=True)
                        ch.append((ps[:, :n], ec, n))
                    hh_t[tt] = h_pool.tile([128, E], BF16, name="hht", tag="h")
                    layernorm(ch, hh_t[tt], 0, use_g1, use_b1)
                    if tt > 0:
                        emit_htrans(tt - 1)
                emit_htrans(NT_B - 1)

                # ---- FFN + LN2 (full 512-token chunk: N=512 Wi GEMMs) ----
                ffT = [None] * FT
                for ft in range(FT):
                    ps = p_mm.tile([128, 512], F32, name="fips", tag="mm")
                    for k in range(KT_E):
                        nc.tensor.matmul(
                            ps[:, :], WI[k][:, ft * 128:(ft + 1) * 128],
                            hT[:, k * S:k * S + 512],
                            start=(k == 0), stop=(k == KT_E - 1))
                    ffT[ft] = fft_pool.tile([128, 512], BF16, name="fftt",
                                            tag="fft")
                    if use_bi:
                        nc.scalar.activation(ffT[ft][:, :], ps[:, :],
                                             AF.Gelu,
                                             bias=bic[:, ft:ft + 1])
                    else:
                        nc.scalar.activation(ffT[ft][:, :], ps[:, :],
                                             AF.Gelu)
                for tt in range(NT_B):
                    ch = []
                    for ec, n in ((0, 512), (512, 256)):
                        ps = (p_mm.tile([128, 512], F32, name="wops",
                                        tag="mm") if n == 512 else
                              p_mm.tile([128, 256], F32, name="wops2",
                                        tag="mm"))
                        for f in range(FT):
                            nc.tensor.matmul(
                                ps[:, :n],
                                ffT[f][:, tt * 128:(tt + 1) * 128],
                                WOUT[f][:, ec:ec + n],
                                start=(f == 0),
                                stop=(f == FT - 1 and not use_bout))
                        if use_bout:
                            nc.tensor.matmul(
                                ps[:, :n], ones[0:1, 0:128],
                                brow[5:6, ec:ec + n], start=False,
                                stop=True)
                        ch.append((ps[:, :n], ec, n))
                    otile = out_pool.tile([128, E], F32, name="ot",
                                          tag="outp")
                    resid = [hh_t[tt][:, ec:ec + n] for (_, ec, n) in ch]
                    layernorm(ch, otile, 1, use_g2, use_b2, resid=resid)
                    nc.gpsimd.dma_start(
                        d_out[t0 + tt * 128:t0 + (tt + 1) * 128, :],
                        otile[:, :])
    nc.compile()
    return nc


def _get_program(flags):
    key = ("prog", flags)
    if key not in _CACHE:
        _CACHE[key] = _build(flags)
    return _CACHE[key]


def kernel(x, mask, Wq, bq, Wk, bk, Wv, bv, Wo, bo,
           Wso, bso, gso, beso, Wi, bi, Wout, bout, gout, beout):
    from concourse.bass_utils import run_bass_kernel_spmd

    x = np.asarray(x, np.float32)
    mask = np.asarray(mask)
    sc = 1.0 / float(np.sqrt(np.float32(DK)))

    z = lambda a: not np.any(np.asarray(a))
    one = lambda a: bool(np.all(np.asarray(a) == 1.0))
    flags = (not z(bq), not z(bk), not z(bv), not z(bo), not z(bso),
             not z(bi), not z(bout),
             not one(gso), not z(beso), not one(gout), not z(beout))
    nc = _get_program(flags)

    wq_b = _bf(np.asarray(Wq, np.float32) * sc)
    wk_b, wv_b, wo_b, wso_b = _bf(Wk), _bf(Wv), _bf(Wo), _bf(Wso)
    wi_b, wout_b = _bf(Wi), _bf(Wout)
    identb = _bf(np.eye(128))
    onesr = _bf(np.ones((1, 512)))

    brow = np.zeros((7, FF), np.float32)
    brow[0, :E] = np.asarray(bq, np.float32) * sc
    for i, v in enumerate((bk, bv, bo, bso, bout)):
        brow[i + 1, :E] = v
    brow[6, :] = bi
    brow = _bf(brow)
    bicol = np.asarray(bi, np.float32).reshape(FF // 128, 128).T.copy()
    gbt = np.zeros((128, 4 * E), np.float32)
    for i, g in enumerate((gso, gout, beso, beout)):   # gamma1|gamma2|beta1|beta2
        gbt[:, i * E:(i + 1) * E] = np.broadcast_to(
            np.asarray(g, np.float32).reshape(1, E), (128, E))

    in_maps = []
    for c in range(NCORES):
        xs = x[c * BL:(c + 1) * BL].reshape(T, E)
        ms = np.asarray(mask[c * BL:(c + 1) * BL]).reshape(BL, S)
        mbias = _bf(np.where(ms == 0, np.float32(MASK_NEG),
                             np.float32(0.0)).reshape(1, T))
        in_maps.append({
            "x": _bf(xs), "wq": wq_b, "wk": wk_b, "wv": wv_b, "wo": wo_b,
            "wso": wso_b, "wi": wi_b, "wout": wout_b, "mbias": mbias,
            "ident": identb, "onesrow": onesr,
            "brow": brow, "bicol": bicol, "gb": gbt,
        })

    trace = os.environ.get("KERNEL_TRACE", "0") == "1"
    res = run_bass_kernel_spmd(nc, in_maps, core_ids=list(range(NCORES)),
                               trace=trace)
    if trace and res.exec_time_ns is not None:
        print(f"HW exec time: {res.exec_time_ns} ns")
        if res.instructions_and_trace is not None:
            print(f"trace: {res.instructions_and_trace[1]}")
    out = np.concatenate([r["out"].reshape(BL, S, E) for r in res.results],
                         axis=0)
    return np.ascontiguousarray(out.astype(np.float32))
